# revision 1
# baseline (speedup 1.0000x reference)
"""DeepSeekV3-style 2-layer transformer (MLA attention + dense-EP MoE) on 8 Trainium2 cores.

Sharding (per core c of 8):
  - attention head-parallel: core c owns heads {2c, 2c+1}; shared down-projections
    replicated; partial ao @ Wo[rows] summed via AllReduce.
  - MoE expert-parallel: core c runs expert e=c densely over all 512 tokens and scales
    by the top-2 gate column wtok[:, c] (zero when unselected -> mathematically exact);
    shared expert split over the FFN dim (512 cols/core); one AllReduce combines.
  - LM head vocab-split (4000 cols/core), concatenated on host.

On-chip layout: residual stream kept feature-major x_t [H, T] so weights feed matmuls
as stationary operands in natural [in, out] layout. LayerNorm stats via ones-matmul
partition reductions + rank-1 broadcast matmuls. RoPE via host permutation of Wuk/Wuq
output columns into [even|odd] halves (scores are invariant to a shared q/k dim
permutation; v & Wo stay natural). Softmax without max subtraction (scores are O(1);
masked lanes get -1e9 pre-scale -> exp underflows to 0), denominators via a ones
column appended to v. Matmuls run in float32r (full rate at N>=256, ~1e-4 rounding).
"""

import os
import sys
from contextlib import ExitStack

for _p in ("/opt/trn_rl_repo", "/root/.axon_site/_ro/trn_rl_repo"):
    if os.path.isdir(_p) and _p not in sys.path:
        sys.path.insert(0, _p)

import ml_dtypes
import numpy as np

import concourse.bass as bass
import concourse.tile as tile
from concourse import bacc, mybir
from concourse.bass import ts
from concourse.bass_utils import run_bass_kernel_spmd

F32 = mybir.dt.float32
F32R = mybir.dt.float32r
BF16 = mybir.dt.bfloat16
AF = mybir.ActivationFunctionType
OP = mybir.AluOpType

V, H, NH, HD, CKV, CQ, E, TOPK, L, B, S = 32000, 1024, 16, 64, 256, 384, 8, 2, 2, 1, 512
F = 4 * H
NCORE = 8
VSH = V // NCORE          # 4000
FSH = F // NCORE          # 512
HT = H // 128             # 8 feature tiles
TT = S // 128             # 4 token chunks
HPC = NH // NCORE         # 2 heads per core
EPS = 1e-5
FB = 256                  # F-block width for expert streaming (2 f-tiles)
VB = 512                  # vocab block width for LM head

_CACHE = {}


def _build(causal: bool):
    nc = bacc.Bacc("TRN2", target_bir_lowering=False, debug=False, num_devices=NCORE)

    d = {}

    def din(name, shape, dt):
        d[name] = nc.dram_tensor(name, list(shape), dt, kind="ExternalInput")
        return d[name]

    din("x0t", [H, S], F32R)
    din("cos64", [64, S], F32)
    din("sin64", [64, S], F32)
    din("onescol", [128, 1], F32R)
    din("onesrow", [1, 128], F32R)
    din("onesrowb", [1, 128], BF16)
    din("onesrow32", [1, 128], F32)
    din("onestok", [128, 1], F32R)
    din("ident", [128, 128], F32)
    din("expsel", [E, 128], F32R)
    if causal:
        din("diag", [128, 128], F32)
    else:
        din("maskbt", [S, S], F32)
    ccols = []
    for l in range(L):
        for nm, shape in (
            (f"ln1g_{l}", [128, HT]), (f"ln1b_{l}", [128, HT]),
            (f"ln2g_{l}", [128, HT]), (f"ln2b_{l}", [128, HT]),
            (f"bdkv_{l}", [128, CKV // 128]), (f"bdq_{l}", [128, CQ // 128]),
            (f"buk_{l}", [64, 2]), (f"buq_{l}", [64, 2]),
            (f"bo_{l}", [128, HT]), (f"sb1_{l}", [128, FSH // 128]),
            (f"sb2_{l}", [128, HT]), (f"eb1_{l}", [128, F // 128]),
            (f"eb2_{l}", [128, HT]),
        ):
            din(nm, shape, F32)
            ccols.append(nm)
        din(f"wdkv_{l}", [H, CKV], F32R)
        din(f"wdq_{l}", [H, CQ], F32R)
        din(f"wuk_{l}", [CKV, 128], F32R)
        din(f"wuq_{l}", [CQ, 128], F32R)
        din(f"wuv_{l}", [CKV, 128], F32R)
        din(f"buvr_{l}", [1, 128], F32R)
        din(f"wo_{l}", [128, H], F32R)
        din(f"gatew_{l}", [H, E], F32R)
        din(f"gatebr_{l}", [1, E], F32R)
        din(f"sw1_{l}", [H, FSH], F32R)
        din(f"sw2_{l}", [FSH, H], F32R)
        din(f"ew1_{l}", [H, F], F32R)
        din(f"ew2_{l}", [F, H], F32R)
    for nm in ("flng", "flnb"):
        din(nm, [128, HT], F32)
        ccols.append(nm)
    din("outw", [H, VSH], BF16)
    din("outbr", [1, VSH], BF16)
    out = nc.dram_tensor("out", [S, VSH], F32, kind="ExternalOutput")

    rgroups = [list(range(NCORE))]
    TH = S // 2  # 256, T-half width

    with ExitStack() as ex, nc.allow_low_precision(reason="f32r matmul pipeline"):
        tc = ex.enter_context(tile.TileContext(nc))
        cpool = ex.enter_context(tc.tile_pool(name="const", bufs=1))
        xpool = ex.enter_context(tc.tile_pool(name="xres", bufs=1))
        npool = ex.enter_context(tc.tile_pool(name="normed", bufs=1))
        wpool = ex.enter_context(tc.tile_pool(name="weights", bufs=1))
        apool = ex.enter_context(tc.tile_pool(name="acts", bufs=1))
        ppool = ex.enter_context(tc.tile_pool(name="psum", bufs=1, space="PSUM"))
        dpool = ex.enter_context(tc.tile_pool(name="dram", bufs=1, space="DRAM"))

        # per-boundary, per-T-half bounce buffers
        ar_in = [[dpool.tile([H, TH], F32, name=f"arin{i}_{th}") for th in range(2)]
                 for i in range(2 * L)]
        ar_out = [[dpool.tile([H, TH], F32, name=f"arout{i}_{th}", addr_space="Shared")
                   for th in range(2)] for i in range(2 * L)]

        # ---------------- constants ----------------
        def cload(nm, dt=F32):
            t = cpool.tile(list(d[nm].shape), dt, name=f"c_{nm}")
            nc.sync.dma_start(out=t[:], in_=d[nm][:])
            return t

        xt_h = [[xpool.tile([128, TH], F32R, name=f"xres{th}_{k}") for k in range(HT)]
                for th in range(2)]
        for th in range(2):
            for k in range(HT):
                nc.sync.dma_start(
                    out=xt_h[th][k][:],
                    in_=d["x0t"][:].rearrange("(a p) t -> p a t", p=128)[:, k, th * TH:(th + 1) * TH])
        onescol = cload("onescol", F32R)
        onesrow = cload("onesrow", F32R)
        onesrowb = cload("onesrowb", BF16)
        onesrow32 = cload("onesrow32")
        cos64 = cload("cos64")
        sin64 = cload("sin64")
        ident = cload("ident")
        expsel = cload("expsel", F32R)
        if causal:
            diag = cload("diag")
        else:
            maskbt = cpool.tile([128, TT, S], F32, name="c_maskbt")
            nc.sync.dma_start(out=maskbt[:],
                              in_=d["maskbt"][:].rearrange("(c p) q -> p c q", p=128))
        cc = {nm: cload(nm) for nm in ccols}

        # ---------------- helpers (all per T-half) ----------------
        def layer_norm(th, gc, bcol, dst_fn):
            """Per-token LN of xt_h[th]; writes dst_fn(k) <- normalized [128, TH]."""
            xth = xt_h[th]
            ps_sum = ppool.tile([1, TH], F32, name="psU", bufs=3)
            ps_sq = ppool.tile([1, TH], F32, name="psU", bufs=3)
            for k in range(HT):
                nc.tensor.matmul(ps_sum[:], onescol[:], xth[k][:],
                                 start=(k == 0), stop=(k == HT - 1))
            for k in range(HT):
                sq = apool.tile([128, TH], F32R, name="ln_sq", bufs=2)
                nc.scalar.square(sq[:], xth[k][:].bitcast(F32))
                nc.tensor.matmul(ps_sq[:], onescol[:], sq[:],
                                 start=(k == 0), stop=(k == HT - 1))
            mrow = apool.tile([1, TH], F32, name="ln_mrow", bufs=1)
            nc.scalar.activation(mrow[:], ps_sum[:], AF.Copy, bias=0.0, scale=1.0 / H)
            sqrow = apool.tile([1, TH], F32, name="ln_sqrow", bufs=1)
            nc.scalar.activation(sqrow[:], ps_sq[:], AF.Copy, bias=0.0, scale=1.0 / H)
            msq = apool.tile([1, TH], F32, name="ln_msq", bufs=1)
            nc.vector.tensor_tensor(out=msq[:], in0=mrow[:], in1=mrow[:], op=OP.mult)
            nc.vector.tensor_tensor(out=sqrow[:], in0=sqrow[:], in1=msq[:], op=OP.subtract)
            nc.vector.tensor_scalar(out=sqrow[:], in0=sqrow[:], scalar1=float(EPS),
                                    scalar2=None, op0=OP.add)
            # rsqrt(v) = exp(-0.5 * ln(v)) — keeps ACT in the ln/exp table set
            nc.scalar.activation(sqrow[:], sqrow[:], AF.Ln, bias=0.0, scale=1.0)
            rsrow = apool.tile([1, TH], F32, name="ln_rsrow", bufs=1)
            nc.scalar.activation(rsrow[:], sqrow[:], AF.Exp, bias=0.0, scale=-0.5)
            nc.vector.tensor_tensor(out=mrow[:], in0=mrow[:], in1=rsrow[:], op=OP.mult)
            bc_rs = ppool.tile([128, TH], F32, name="psU", bufs=3)
            nc.tensor.matmul(bc_rs[:], onesrow32[:], rsrow[:], start=True, stop=True)
            bc_mrs = ppool.tile([128, TH], F32, name="psU", bufs=3)
            nc.tensor.matmul(bc_mrs[:], onesrow32[:], mrow[:], start=True, stop=True)
            for k in range(HT):
                t = apool.tile([128, TH], F32, name="ln_t", bufs=2)
                nc.vector.tensor_tensor(out=t[:], in0=xth[k][:].bitcast(F32),
                                        in1=bc_rs[:], op=OP.mult)
                nc.vector.tensor_tensor(out=t[:], in0=t[:], in1=bc_mrs[:], op=OP.subtract)
                nc.scalar.activation(dst_fn(k), t[:], AF.Identity,
                                     bias=bcol[:, k:k + 1], scale=gc[:, k:k + 1])

        def x_update(th, ar_out_t, bias_cc):
            for k in range(HT):
                dst_ = apool.tile([128, TH], F32, name="delta_st", bufs=3)
                nc.sync.dma_start(
                    out=dst_[:],
                    in_=ar_out_t[:].rearrange("(a p) t -> p a t", p=128)[:, k, :])
                nc.vector.scalar_tensor_tensor(
                    out=xt_h[th][k][:], in0=dst_[:], scalar=bias_cc[:, k:k + 1],
                    in1=xt_h[th][k][:].bitcast(F32), op0=OP.add, op1=OP.add)

        for l in range(L):
            # ======================= attention (per T-half pipeline) =======================
            wuk = wpool.tile([128, 2, 128], F32R, name="wuk")
            nc.sync.dma_start(out=wuk[:], in_=d[f"wuk_{l}"][:].rearrange("(a p) c -> p a c", p=128))
            wuq = wpool.tile([128, 3, 128], F32R, name="wuq")
            nc.sync.dma_start(out=wuq[:], in_=d[f"wuq_{l}"][:].rearrange("(a p) c -> p a c", p=128))
            wuv = wpool.tile([128, 2, 128], F32R, name="wuv")
            nc.sync.dma_start(out=wuv[:], in_=d[f"wuv_{l}"][:].rearrange("(a p) c -> p a c", p=128))
            buvr = wpool.tile([1, 128], F32R, name="buvr")
            nc.sync.dma_start(out=buvr[:], in_=d[f"buvr_{l}"][:])

            def attn_part2(th):
                t0 = th * TH
                # scores/exp/av for q-tokens in this half (causal: k-chunks <= half end)
                ao_all = apool.tile([128, TH], F32R, name=f"ao{th}")
                ntk = 2 * (th + 1) if causal else TT
                for hi in range(HPC):
                    r = 64 * hi
                    ao_ps = ppool.tile([65, TH], F32, name="psE", bufs=2)
                    for tk in range(ntk):
                        n0 = max(128 * tk - t0, 0) if causal else 0
                        kf = kfin_h[tk // 2]
                        ps_u = ppool.tile([128, TH], F32, name="psU", bufs=3)
                        nc.tensor.matmul(ps_u[:, n0:TH], kf[r:r + 64, ts(tk % 2, 128)],
                                         qfin_h[th][r:r + 64, n0:TH], start=True, stop=True)
                        if causal and 128 * tk >= t0:
                            nc.vector.tensor_tensor(out=ps_u[:, n0:n0 + 128],
                                                    in0=ps_u[:, n0:n0 + 128], in1=diag[:],
                                                    op=OP.add)
                        elif not causal:
                            nc.vector.tensor_tensor(out=ps_u[:, :], in0=ps_u[:, :],
                                                    in1=maskbt[:, tk, t0:t0 + TH], op=OP.add)
                        u = apool.tile([128, TH], F32R, name="u_exp", bufs=2)
                        nc.scalar.activation(u[:, n0:TH], ps_u[:, n0:TH], AF.Exp,
                                             bias=0.0, scale=1.0 / np.sqrt(HD))
                        nc.tensor.matmul(ao_ps[:, n0:TH],
                                         vt_h[tk // 2][:, tk % 2, 65 * hi:65 * hi + 65],
                                         u[:, n0:TH], start=(tk == 0), stop=(tk == ntk - 1),
                                         skip_group_check=True)
                    rrow = apool.tile([1, TH], F32, name="rrow", bufs=1)
                    nc.vector.reciprocal(out=rrow[:], in_=ao_ps[64:65, :])
                    bcp = ppool.tile([64, TH], F32, name="psU", bufs=3)
                    nc.tensor.matmul(bcp[:], onesrow32[:, 0:64], rrow[:], start=True, stop=True)
                    bcs = apool.tile([64, TH], F32, name="bcs", bufs=1)
                    nc.scalar.copy(bcs[:], bcp[:])
                    nc.vector.tensor_tensor(out=ao_all[r:r + 64, :], in0=ao_ps[0:64, :],
                                            in1=bcs[:], op=OP.mult)
                ao_h.append(ao_all)

                # Wo partial for this half -> AR half
                wo = wpool.tile([128, HT, 128], F32R, name="wproj", bufs=2)
                nc.sync.dma_start(out=wo[:],
                                  in_=d[f"wo_{l}"][:].rearrange("p (a q) -> p a q", q=128))
                for a in range(HT):
                    ps = ppool.tile([128, TH], F32, name="psA", bufs=3)
                    nc.tensor.matmul(ps[:], wo[:, a, :], ao_all[:], start=True, stop=True)
                    st = apool.tile([128, TH], F32, name="evac_st2", bufs=3)
                    nc.scalar.copy(st[:], ps[:])
                    nc.sync.dma_start(
                        out=ar_in[2 * l][th][:].rearrange("(a p) t -> p a t", p=128)[:, a, :],
                        in_=st[:])
                nc.gpsimd.collective_compute(
                    "AllReduce", OP.add, replica_groups=rgroups,
                    ins=[ar_in[2 * l][th][:]], outs=[ar_out[2 * l][th][:]])


            kfin_h, qfin_h, vt_h, ao_h = [], [], [], []
            for th in range(2):
                t0 = th * TH
                if l > 0:
                    x_update(th, ar_out[2 * l - 1][th], cc[f"sb2_{l - 1}"])
                ht_th = [npool.tile([128, TH], F32R, name=f"nrm{th}_{k}") for k in range(HT)]
                layer_norm(th, cc[f"ln1g_{l}"], cc[f"ln1b_{l}"], lambda k: ht_th[k][:])

                kvt = apool.tile([128, CKV // 128, TH], F32R, name=f"kvt{th}")
                for c in range(CKV // 128):
                    wblk = wpool.tile([128, HT, 128], F32R, name="wproj", bufs=2)
                    nc.sync.dma_start(
                        out=wblk[:],
                        in_=d[f"wdkv_{l}"][:].rearrange("(a p) c -> p a c", p=128)[:, :, ts(c, 128)])
                    ps = ppool.tile([128, TH], F32, name="psA", bufs=3)
                    for a in range(HT):
                        nc.tensor.matmul(ps[:], wblk[:, a, :], ht_th[a][:],
                                         start=(a == 0), stop=(a == HT - 1))
                    nc.scalar.activation(kvt[:, c, :], ps[:], AF.Identity,
                                         bias=cc[f"bdkv_{l}"][:, c:c + 1], scale=1.0)
                qdt = apool.tile([128, CQ // 128, TH], F32R, name=f"qdt{th}")
                for c in range(CQ // 128):
                    wblk = wpool.tile([128, HT, 128], F32R, name="wproj", bufs=2)
                    nc.sync.dma_start(
                        out=wblk[:],
                        in_=d[f"wdq_{l}"][:].rearrange("(a p) c -> p a c", p=128)[:, :, ts(c, 128)])
                    ps = ppool.tile([128, TH], F32, name="psA", bufs=3)
                    for a in range(HT):
                        nc.tensor.matmul(ps[:], wblk[:, a, :], ht_th[a][:],
                                         start=(a == 0), stop=(a == HT - 1))
                    nc.scalar.activation(qdt[:, c, :], ps[:], AF.Identity,
                                         bias=cc[f"bdq_{l}"][:, c:c + 1], scale=1.0)

                def rope_from_psum(ps, bias32, dst):
                    cosv = cos64[:, t0:t0 + TH]
                    sinv = sin64[:, t0:t0 + TH]
                    x1, x2 = ps[0:64, :], ps[64:128, :]
                    b1, b2 = bias32[:, 0:1], bias32[:, 1:2]
                    ta_ = apool.tile([64, TH], F32, name="rope_t", bufs=2)
                    nc.vector.scalar_tensor_tensor(out=ta_[:], in0=x1, scalar=b1,
                                                   in1=cosv, op0=OP.add, op1=OP.mult)
                    tb_ = apool.tile([64, TH], F32, name="rope_t", bufs=2)
                    nc.vector.scalar_tensor_tensor(out=tb_[:], in0=x2, scalar=b2,
                                                   in1=sinv, op0=OP.add, op1=OP.mult)
                    nc.vector.tensor_tensor(out=dst[0:32, :], in0=ta_[0:32, :],
                                            in1=tb_[0:32, :], op=OP.subtract)
                    nc.vector.tensor_tensor(out=dst[64:96, :], in0=ta_[32:64, :],
                                            in1=tb_[32:64, :], op=OP.subtract)
                    tc_ = apool.tile([64, TH], F32, name="rope_t", bufs=2)
                    nc.vector.scalar_tensor_tensor(out=tc_[:], in0=x1, scalar=b1,
                                                   in1=sinv, op0=OP.add, op1=OP.mult)
                    td_ = apool.tile([64, TH], F32, name="rope_t", bufs=2)
                    nc.vector.scalar_tensor_tensor(out=td_[:], in0=x2, scalar=b2,
                                                   in1=cosv, op0=OP.add, op1=OP.mult)
                    nc.vector.tensor_tensor(out=dst[32:64, :], in0=tc_[0:32, :],
                                            in1=td_[0:32, :], op=OP.add)
                    nc.vector.tensor_tensor(out=dst[96:128, :], in0=tc_[32:64, :],
                                            in1=td_[32:64, :], op=OP.add)

                kfin = apool.tile([128, TH], F32R, name=f"kfin{th}")
                ps_k = ppool.tile([128, TH], F32, name="psA", bufs=3)
                for c in range(CKV // 128):
                    nc.tensor.matmul(ps_k[:], wuk[:, c, :], kvt[:, c, :],
                                     start=(c == 0), stop=(c == CKV // 128 - 1))
                rope_from_psum(ps_k, cc[f"buk_{l}"], kfin)
                kfin_h.append(kfin)

                qfin = apool.tile([128, TH], F32R, name=f"qfin{th}")
                ps_q = ppool.tile([128, TH], F32, name="psA", bufs=3)
                for c in range(CQ // 128):
                    nc.tensor.matmul(ps_q[:], wuq[:, c, :], qdt[:, c, :],
                                     start=(c == 0), stop=(c == CQ // 128 - 1))
                rope_from_psum(ps_q, cc[f"buq_{l}"], qfin)
                qfin_h.append(qfin)

                vt = apool.tile([128, 2, 130], F32R, name=f"vt{th}")
                for tl in range(2):
                    ps = ppool.tile([128, 128], F32, name="psA", bufs=3)
                    nc.tensor.matmul(ps[:], onesrow[:], buvr[:], start=True, stop=False)
                    for c in range(CKV // 128):
                        nc.tensor.matmul(ps[:], kvt[:, c, ts(tl, 128)], wuv[:, c, :],
                                         start=False, stop=(c == CKV // 128 - 1))
                    nc.scalar.copy(vt[:, tl, 0:64], ps[:, 0:64])
                    nc.scalar.copy(vt[:, tl, 65:129], ps[:, 64:128])
                    nc.sync.dma_start(out=vt[:, tl, 64:65], in_=d["onestok"][:])
                    nc.sync.dma_start(out=vt[:, tl, 129:130], in_=d["onestok"][:])
                vt_h.append(vt)
                if causal:
                    attn_part2(th)

            if not causal:
                for th in range(2):
                    attn_part2(th)

            # ======================= MoE =======================
            hmt = [npool.tile([128, S], F32R, name=f"hmt{k}") for k in range(HT)]
            for th in range(2):
                t0 = th * TH
                x_update(th, ar_out[2 * l][th], cc[f"bo_{l}"])
                layer_norm(th, cc[f"ln2g_{l}"], cc[f"ln2b_{l}"],
                           lambda k, _t0=t0: hmt[k][:, _t0:_t0 + TH])

            # gate + top-2 weights (cheap; after both halves)
            gatew = wpool.tile([128, HT, E], F32R, name="gatew")
            nc.sync.dma_start(out=gatew[:], in_=d[f"gatew_{l}"][:].rearrange("(a p) e -> p a e", p=128))
            gatebr = wpool.tile([1, E], F32R, name="gatebr")
            nc.sync.dma_start(out=gatebr[:], in_=d[f"gatebr_{l}"][:])
            wb = apool.tile([128, S], F32, name="wb", bufs=2)
            for th in range(2):
                t0 = th * TH
                wtok = apool.tile([128, 2, E], F32, name="wtok", bufs=2)
                for tl in range(2):
                    t = 2 * th + tl
                    ps = ppool.tile([128, E], F32, name="psA", bufs=3)
                    nc.tensor.matmul(ps[:], onesrow[:], gatebr[:], start=True, stop=False)
                    for a in range(HT):
                        nc.tensor.matmul(ps[:], hmt[a][:, ts(t, 128)], gatew[:, a, :],
                                         start=False, stop=(a == HT - 1))
                    z = apool.tile([128, E], F32, name="gz", bufs=1)
                    nc.scalar.copy(z[:], ps[:])
                    m1 = apool.tile([128, 1], F32, name="gm1", bufs=2)
                    nc.vector.tensor_reduce(out=m1[:], in_=z[:], axis=mybir.AxisListType.X, op=OP.max)
                    nm1 = apool.tile([128, 1], F32, name="gnm1", bufs=2)
                    nc.vector.tensor_scalar(out=nm1[:], in0=m1[:], scalar1=-1.0,
                                            scalar2=None, op0=OP.mult)
                    eqm = apool.tile([128, E], F32, name="geqm", bufs=1)
                    nc.vector.tensor_scalar(out=eqm[:], in0=z[:], scalar1=m1[:],
                                            scalar2=None, op0=OP.is_ge)
                    zm = apool.tile([128, E], F32, name="gzm", bufs=1)
                    nc.vector.scalar_tensor_tensor(out=zm[:], in0=eqm[:], scalar=-1e30,
                                                   in1=z[:], op0=OP.mult, op1=OP.add)
                    m2 = apool.tile([128, 1], F32, name="gm2", bufs=2)
                    nc.vector.tensor_reduce(out=m2[:], in_=zm[:], axis=mybir.AxisListType.X, op=OP.max)
                    sel = apool.tile([128, E], F32, name="gsel", bufs=1)
                    nc.vector.tensor_scalar(out=sel[:], in0=z[:], scalar1=m2[:],
                                            scalar2=None, op0=OP.is_ge)
                    u8 = apool.tile([128, E], F32, name="gu8", bufs=1)
                    nc.scalar.activation(u8[:], z[:], AF.Exp, bias=nm1[:], scale=1.0)
                    em2 = apool.tile([128, 1], F32, name="gem2", bufs=2)
                    nc.scalar.activation(em2[:], m2[:], AF.Exp, bias=nm1[:], scale=1.0)
                    den = apool.tile([128, 1], F32, name="gden", bufs=2)
                    nc.vector.tensor_scalar(out=den[:], in0=em2[:], scalar1=1.0,
                                            scalar2=None, op0=OP.add)
                    rec = apool.tile([128, 1], F32, name="grec", bufs=2)
                    nc.vector.reciprocal(out=rec[:], in_=den[:])
                    us = apool.tile([128, E], F32, name="gus", bufs=1)
                    nc.vector.tensor_tensor(out=us[:], in0=u8[:], in1=sel[:], op=OP.mult)
                    nc.vector.tensor_scalar(out=wtok[:, tl, :], in0=us[:], scalar1=rec[:],
                                            scalar2=None, op0=OP.mult)
                ps_wt = ppool.tile([E, TH], F32, name="psA", bufs=3)
                for tl in range(2):
                    nc.tensor.transpose(ps_wt[:, ts(tl, 128)], wtok[:, tl, :], ident[:])
                wt_sb = apool.tile([E, TH], F32R, name="wt_sb", bufs=1)
                nc.scalar.copy(wt_sb[:], ps_wt[:])
                ps_wb = ppool.tile([128, TH], F32, name="psA", bufs=3)
                nc.tensor.matmul(ps_wb[:], expsel[:], wt_sb[:], start=True, stop=True)
                nc.scalar.copy(wb[:, t0:t0 + TH], ps_wb[:])

            # expert: f32r streamed in 256-wide F pairs; W1 -> gelu -> W2 -> SBUF accumulate
            moe_acc = apool.tile([128, HT, S], F32, name="moe_acc")

            def ffn_pair(w1_src, w2_src, b1cc, b1ofs, fp, first):
                """One 512-wide F stripe (two 256 sub-blocks): W1+gelu then W2 sweeps.
                Returns nothing; accumulates into moe_acc (tensor add unless first)."""
                h1sub, w2sub = [], []
                for sub in range(2):
                    fb = 2 * fp + sub
                    w1blk = wpool.tile([128, HT, 256], F32R, name="w1blk", bufs=2)
                    nc.sync.dma_start(
                        out=w1blk[:],
                        in_=w1_src[:].rearrange("(a p) f -> p a f", p=128)[:, :, ts(fb, 256)])
                    h1blk = apool.tile([128, 2, S], F32R, name="h1blk", bufs=2)
                    for f4 in range(2):
                        f = b1ofs + 2 * fb + f4
                        ps = ppool.tile([128, S], F32, name="psA", bufs=3)
                        for a in range(HT):
                            nc.tensor.matmul(ps[:], w1blk[:, a, ts(f4, 128)], hmt[a][:],
                                             start=(a == 0), stop=(a == HT - 1))
                        nc.scalar.activation(h1blk[:, f4, :], ps[:], AF.Gelu,
                                             bias=b1cc[:, f:f + 1], scale=1.0)
                    h1sub.append(h1blk)
                    w2blk = wpool.tile([128, 2, H], F32R, name="w2blk", bufs=2)
                    nc.sync.dma_start(
                        out=w2blk[:],
                        in_=w2_src[:].rearrange("(g p) h -> p g h", p=128)[:, 2 * fb:2 * fb + 2, :])
                    w2sub.append(w2blk)
                for hh in range(HT):
                    ps = ppool.tile([128, S], F32, name="psE", bufs=2)
                    for sub in range(2):
                        for f4 in range(2):
                            nc.tensor.matmul(ps[:], w2sub[sub][:, f4, ts(hh, 128)],
                                             h1sub[sub][:, f4, :],
                                             start=(sub == 0 and f4 == 0),
                                             stop=(sub == 1 and f4 == 1))
                    if first:
                        nc.vector.tensor_copy(out=moe_acc[:, hh, :], in_=ps[:])
                    else:
                        nc.vector.tensor_tensor(out=moe_acc[:, hh, :],
                                                in0=moe_acc[:, hh, :], in1=ps[:], op=OP.add)

            for fp in range(F // 512):
                ffn_pair(d[f"ew1_{l}"], d[f"ew2_{l}"], cc[f"eb1_{l}"], 0, fp, fp == 0)
            # (eo + eb2) * wtok
            for hh in range(HT):
                nc.vector.scalar_tensor_tensor(
                    out=moe_acc[:, hh, :], in0=moe_acc[:, hh, :],
                    scalar=cc[f"eb2_{l}"][:, hh:hh + 1], in1=wb[:],
                    op0=OP.add, op1=OP.mult)
            # shared expert slice: one more 512-wide stripe added in
            ffn_pair(d[f"sw1_{l}"], d[f"sw2_{l}"], cc[f"sb1_{l}"], 0, 0, False)

            # per T-half: stage out -> AllReduce
            for th in range(2):
                t0 = th * TH
                nc.sync.dma_start(
                    out=ar_in[2 * l + 1][th][:].rearrange("(a p) t -> p a t", p=128),
                    in_=moe_acc[:, :, t0:t0 + TH])
                nc.gpsimd.collective_compute(
                    "AllReduce", OP.add, replica_groups=rgroups,
                    ins=[ar_in[2 * l + 1][th][:]], outs=[ar_out[2 * l + 1][th][:]])

        # ======================= final LN + LM head =======================
        xft_h = []
        for th in range(2):
            x_update(th, ar_out[2 * L - 1][th], cc[f"sb2_{L - 1}"])
            xft = [npool.tile([128, TH], BF16, name=f"nrm{th}_{k}") for k in range(HT)]
            layer_norm(th, cc["flng"], cc["flnb"], lambda k: xft[k][:])
            xft_h.append(xft)

        nvb = (VSH + VB - 1) // VB
        for tpass in ((0, 1), (2, 3)):
            for vb in range(nvb):
                vn = min(VB, VSH - VB * vb)
                owblk = wpool.tile([128, HT, VB], BF16, name="owblk", bufs=2)
                nc.sync.dma_start(
                    out=owblk[:, :, 0:vn],
                    in_=d["outw"][:].rearrange("(a p) v -> p a v", p=128)[:, :, VB * vb:VB * vb + vn])
                obr = wpool.tile([1, VB], BF16, name="outbr_blk", bufs=2)
                nc.sync.dma_start(out=obr[0:1, 0:vn], in_=d["outbr"][0:1, VB * vb:VB * vb + vn])
                for t in tpass:
                    ps = ppool.tile([128, VB], F32, name="psA", bufs=3)
                    nc.tensor.matmul(ps[:, 0:vn], onesrowb[:], obr[0:1, 0:vn],
                                     start=True, stop=False)
                    for a in range(HT):
                        nc.tensor.matmul(ps[:, 0:vn], xft_h[t // 2][a][:, ts(t % 2, 128)],
                                         owblk[:, a, 0:vn], start=False, stop=(a == HT - 1))
                    osb = apool.tile([128, VB], F32, name="evac_st", bufs=2)
                    nc.vector.tensor_copy(out=osb[:, 0:vn], in_=ps[:, 0:vn])
                    nc.sync.dma_start(out=out[ts(t, 128), VB * vb:VB * vb + vn],
                                      in_=osb[:, 0:vn])

    nc.compile()
    return nc


def _host_prepare(inputs):
    ids = np.asarray(inputs["input_ids"])[0]
    mask = np.asarray(inputs["attn_mask"])[0, 0]
    causal = bool(np.array_equal(mask, np.tril(np.ones((S, S), dtype=mask.dtype))))

    emb = np.asarray(inputs["emb"], dtype=np.float32)
    x0t = np.ascontiguousarray(emb[ids].T)

    perm = np.concatenate([np.arange(0, HD, 2), np.arange(1, HD, 2)])
    inv = 1.0 / (10000.0 ** (np.arange(0, HD, 2, dtype=np.float32) / HD))
    pos = np.arange(S, dtype=np.float32)
    ang = pos[None, :] * inv[:, None]

    ii, jj = np.meshgrid(np.arange(128), np.arange(128), indexing="ij")

    def colify(v, width=128):
        v = np.asarray(v, dtype=np.float32).reshape(-1)
        return np.ascontiguousarray(v.reshape(-1, width).T)

    f32a = lambda v: np.ascontiguousarray(np.asarray(v, dtype=np.float32))

    common = {
        "x0t": x0t,
        "cos64": np.tile(np.cos(ang).astype(np.float32), (2, 1)),
        "sin64": np.tile(np.sin(ang).astype(np.float32), (2, 1)),
        "onescol": np.ones((128, 1), np.float32),
        "onesrow": np.ones((1, 128), np.float32),
        "onesrowb": np.ones((1, 128), ml_dtypes.bfloat16),
        "onesrow32": np.ones((1, 128), np.float32),
        "onestok": np.ones((128, 1), np.float32),
        "ident": np.eye(128, dtype=np.float32),
        "flng": colify(inputs["fln_g"]), "flnb": colify(inputs["fln_b"]),
    }
    if causal:
        common["diag"] = np.where(ii > jj, np.float32(-1e9), np.float32(0.0))
    else:
        common["maskbt"] = np.where(mask.T == 0, np.float32(-1e9), np.float32(0.0))

    in_maps = []
    for c in range(NCORE):
        m = dict(common)
        m["expsel"] = np.zeros((E, 128), np.float32)
        m["expsel"][c, :] = 1.0
        ev, od = np.arange(0, HD, 2), np.arange(1, HD, 2)
        pcols = np.concatenate([(2 * c) * HD + ev, (2 * c + 1) * HD + ev,
                                (2 * c) * HD + od, (2 * c + 1) * HD + od])
        ncols = np.arange(128 * c, 128 * c + 128)
        for l in range(L):
            m[f"ln1g_{l}"] = colify(inputs["ln1_g"][l])
            m[f"ln1b_{l}"] = colify(inputs["ln1_b"][l])
            m[f"ln2g_{l}"] = colify(inputs["ln2_g"][l])
            m[f"ln2b_{l}"] = colify(inputs["ln2_b"][l])
            m[f"wdkv_{l}"] = f32a(inputs["Wdkv"][l])
            m[f"bdkv_{l}"] = colify(inputs["bdkv"][l])
            m[f"wdq_{l}"] = f32a(inputs["Wdq"][l])
            m[f"bdq_{l}"] = colify(inputs["bdq"][l])
            m[f"wuk_{l}"] = f32a(np.asarray(inputs["Wuk"][l])[:, pcols])
            m[f"buk_{l}"] = colify(np.asarray(inputs["buk"][l])[pcols], 64)
            m[f"wuq_{l}"] = f32a(np.asarray(inputs["Wuq"][l])[:, pcols])
            m[f"buq_{l}"] = colify(np.asarray(inputs["buq"][l])[pcols], 64)
            m[f"wuv_{l}"] = f32a(np.asarray(inputs["Wuv"][l])[:, ncols])
            m[f"buvr_{l}"] = f32a(np.asarray(inputs["buv"][l])[ncols])[None, :]
            m[f"wo_{l}"] = f32a(np.asarray(inputs["Wo"][l])[ncols, :])
            m[f"bo_{l}"] = colify(inputs["bo"][l])
            m[f"gatew_{l}"] = f32a(inputs["gate_W"][l])
            m[f"gatebr_{l}"] = f32a(np.asarray(inputs["gate_b"][l])
                                    + np.asarray(inputs["gate_bias"][l]))[None, :]
            m[f"sw1_{l}"] = f32a(np.asarray(inputs["sW1"][l])[:, FSH * c:FSH * (c + 1)])
            m[f"sb1_{l}"] = colify(np.asarray(inputs["sb1"][l])[FSH * c:FSH * (c + 1)])
            m[f"sw2_{l}"] = f32a(np.asarray(inputs["sW2"][l])[FSH * c:FSH * (c + 1), :])
            m[f"sb2_{l}"] = colify(inputs["sb2"][l])
            m[f"ew1_{l}"] = f32a(inputs["eW1"][l, c])
            m[f"eb1_{l}"] = colify(inputs["eb1"][l, c])
            m[f"ew2_{l}"] = f32a(inputs["eW2"][l, c])
            m[f"eb2_{l}"] = colify(inputs["eb2"][l, c])
        m["outw"] = f32a(np.asarray(inputs["out_W"])[:, VSH * c:VSH * (c + 1)]).astype(ml_dtypes.bfloat16)
        m["outbr"] = f32a(np.asarray(inputs["out_b"])[VSH * c:VSH * (c + 1)])[None, :].astype(ml_dtypes.bfloat16)
        in_maps.append(m)
    return in_maps, causal


def kernel(**inputs):
    if "attn_mask" not in inputs and "attention_mask" in inputs:
        inputs = dict(inputs)
        inputs["attn_mask"] = inputs.pop("attention_mask")
    in_maps, causal = _host_prepare(inputs)
    if causal not in _CACHE:
        _CACHE[causal] = _build(causal)
    nc = _CACHE[causal]
    res = run_bass_kernel_spmd(nc, in_maps, core_ids=list(range(NCORE)))
    shards = [res.results[c]["out"] for c in range(NCORE)]
    return np.concatenate(shards, axis=1)[None].astype(np.float32)



# revision 15
# speedup vs baseline: 1.0245x; 1.0245x over previous
"""DeepSeekV3-style 2-layer transformer (MLA attention + dense-EP MoE) on 8 Trainium2 cores.

v2 sharding (per core c of 8):
  - Attention fully REPLICATED (all 16 heads on every core): no collective at the
    attention boundary; attention output is added to the local residual directly.
  - MoE expert-parallel: core c runs expert e=c densely over all 512 tokens, scaled
    by the top-2 gate column wtok[:, c] (zero when unselected -> exact); shared
    expert split over the FFN dim (512 cols/core). One ReduceScatter+AllGather
    (bf16 payload) per layer combines the partial deltas -- 2 collectives total.
  - LM head vocab-split (4000 cols/core), concatenated on host.

On-chip layout: residual x kept feature-major [H, T] f32r. LayerNorm via ones-matmul
partition reductions + rank-1 broadcast matmuls. RoPE via host permutation of the
k/q up-projection columns into per-head quadrant-local (x1;x2) pairs so the rotation
is 4 wide ops per 128-row tile (2x STT, stream_shuffle, add). Softmax without max
subtraction; denominators via a ones column in v; per-head reciprocal broadcast by
rank-1 matmul. MoE W2 accumulates 8 F-blocks per psum bank (4 F-groups x two 4-high
output passes) to keep evac traffic low. Matmuls in f32r (moving dim >= 256).
"""

import os
import sys
from contextlib import ExitStack

for _p in ("/opt/trn_rl_repo", "/root/.axon_site/_ro/trn_rl_repo"):
    if os.path.isdir(_p) and _p not in sys.path:
        sys.path.insert(0, _p)

import ml_dtypes
import numpy as np

import concourse.bass as bass
import concourse.tile as tile
from concourse import bacc, mybir
from concourse.bass import ts
from concourse.bass_utils import run_bass_kernel_spmd

F32 = mybir.dt.float32
F32R = mybir.dt.float32r
BF16 = mybir.dt.bfloat16
AF = mybir.ActivationFunctionType
OP = mybir.AluOpType

V, H, NH, HD, CKV, CQ, E, TOPK, L, B, S = 32000, 1024, 16, 64, 256, 384, 8, 2, 2, 1, 512
F = 4 * H
NCORE = 8
VSH = V // NCORE          # 4000
FSH = F // NCORE          # 512
HT = H // 128             # 8 feature tiles
TT = S // 128             # 4 token chunks
NHP = NH // 2             # 8 head-pairs (one 128-row tile each)
EPS = 1e-5
TH = S // 2               # 256
VB = 256                  # vocab block width for LM head
NFB = F // 128            # 32 expert F-blocks
FG = 8                    # F-blocks per W2 accumulation group (1024 F cols)
SFB = FSH // 128          # 4 shared-expert F-blocks

_CACHE = {}

# stream_shuffle mask: swap 16-row halves within each 32-partition quadrant
_SWAP16 = list(range(16, 32)) + list(range(16))


def _build(causal: bool):
    nc = bacc.Bacc("TRN2", target_bir_lowering=False, debug=False, num_devices=NCORE)

    d = {}

    def din(name, shape, dt):
        d[name] = nc.dram_tensor(name, list(shape), dt, kind="ExternalInput")
        return d[name]

    din("x0t", [H, S], F32R)
    din("ropec", [128, S], F32)
    din("ropes", [128, S], F32)
    din("onescol", [128, 1], F32R)
    din("onesrow", [1, 128], F32R)
    din("onesrowb", [1, 128], BF16)
    din("onesrow32", [1, 128], F32)
    din("ident", [128, 128], F32)
    din("expsel", [E, 128], F32R)
    if causal:
        din("diag", [128, 128], F32)
    else:
        din("maskbt", [S, S], F32)
    ccols = []
    for l in range(L):
        for nm, shape in (
            (f"ln1g_{l}", [128, HT]), (f"ln1b_{l}", [128, HT]),
            (f"ln2g_{l}", [128, HT]), (f"ln2b_{l}", [128, HT]),
            (f"bdkv_{l}", [128, CKV // 128]), (f"bdq_{l}", [128, CQ // 128]),
            (f"buk_{l}", [128, NHP]), (f"buq_{l}", [128, NHP]),
            (f"bo_{l}", [128, HT]), (f"sb1_{l}", [128, SFB]),
            (f"sb2_{l}", [128, HT]), (f"eb1_{l}", [128, NFB]),
            (f"eb2_{l}", [128, HT]),
        ):
            din(nm, shape, F32)
            ccols.append(nm)
        din(f"wdkv_{l}", [H, CKV], F32R)
        din(f"wdq_{l}", [H, CQ], F32R)
        din(f"wuk_{l}", [CKV, NH * HD], F32R)
        din(f"wuq_{l}", [CQ, NH * HD], F32R)
        din(f"wuv_{l}", [CKV, NH * HD], F32R)
        din(f"buvr_{l}", [1, NH * HD], F32R)
        din(f"wo_{l}", [NH * HD, H], F32R)
        din(f"gatew_{l}", [H, E], F32R)
        din(f"gatebr_{l}", [1, E], F32R)
        din(f"sw1_{l}", [H, FSH], F32R)
        din(f"sw2_{l}", [FSH, H], F32R)
        din(f"ew1_{l}", [H, F], F32R)
        din(f"ew2_{l}", [F, H], F32R)
    for nm in ("flng", "flnb"):
        din(nm, [128, HT], F32)
        ccols.append(nm)
    din("outw", [H, VSH], BF16)
    din("outbr", [1, VSH], BF16)
    out = nc.dram_tensor("out", [S, VSH], F32, kind="ExternalOutput")

    rgroups = [list(range(NCORE))]

    with ExitStack() as ex, nc.allow_low_precision(reason="f32r matmul pipeline"):
        tc = ex.enter_context(tile.TileContext(nc))
        cpool = ex.enter_context(tc.tile_pool(name="const", bufs=1))
        xpool = ex.enter_context(tc.tile_pool(name="xres", bufs=1))
        npool = ex.enter_context(tc.tile_pool(name="normed", bufs=1))
        wpool = ex.enter_context(tc.tile_pool(name="weights", bufs=1))
        apool = ex.enter_context(tc.tile_pool(name="acts", bufs=1))
        ppool = ex.enter_context(tc.tile_pool(name="psum", bufs=1, space="PSUM"))
        dpool = ex.enter_context(tc.tile_pool(name="dram", bufs=1, space="DRAM"))

        # per-layer collective staging (bf16 delta payloads)
        arin = [dpool.tile([H, S], BF16, name=f"arin{l}") for l in range(L)]
        rsout = [dpool.tile([128, S], BF16, name=f"rsout{l}") for l in range(L)]
        agout = [dpool.tile([H, S], BF16, name=f"agout{l}", addr_space="Shared")
                 for l in range(L)]

        # ---------------- constants ----------------
        def cload(nm, dt=F32):
            t = cpool.tile(list(d[nm].shape), dt, name=f"c_{nm}")
            nc.sync.dma_start(out=t[:], in_=d[nm][:])
            return t

        xt = [xpool.tile([128, S], F32R, name=f"xres_{k}") for k in range(HT)]
        for k in range(HT):
            nc.sync.dma_start(
                out=xt[k][:],
                in_=d["x0t"][:].rearrange("(a p) t -> p a t", p=128)[:, k, :])
        onescol = cload("onescol", F32R)
        onesrow = cload("onesrow", F32R)
        onesrowb = cload("onesrowb", BF16)
        onesrow32 = cload("onesrow32")
        ropec = cload("ropec")
        ropes = cload("ropes")
        ident = cload("ident")
        expsel = cload("expsel", F32R)
        if causal:
            diag = cload("diag")
        else:
            maskbt = cpool.tile([128, TT, S], F32, name="c_maskbt")
            nc.sync.dma_start(out=maskbt[:],
                              in_=d["maskbt"][:].rearrange("(c p) q -> p c q", p=128))
        cc = {nm: cload(nm) for nm in ccols}

        # ---------------- helpers ----------------
        def layer_norm(th, gc, bcol, dst_fn, dst_dt_note=""):
            """Per-token LN of xt token-half th; writes dst_fn(k) <- normalized [128, TH]."""
            t0 = th * TH
            ps_sum = ppool.tile([1, TH], F32, name="psS", bufs=2)
            ps_sq = ppool.tile([1, TH], F32, name="psS", bufs=2)
            for k in range(HT):
                nc.tensor.matmul(ps_sum[:], onescol[:], xt[k][:, t0:t0 + TH],
                                 start=(k == 0), stop=(k == HT - 1))
            for k in range(HT):
                sq = apool.tile([128, TH], F32R, name="ln_sq", bufs=1)
                nc.scalar.square(sq[:], xt[k][:, t0:t0 + TH].bitcast(F32))
                nc.tensor.matmul(ps_sq[:], onescol[:], sq[:],
                                 start=(k == 0), stop=(k == HT - 1))
            mrow = apool.tile([1, TH], F32, name="ln_mrow", bufs=1)
            nc.scalar.activation(mrow[:], ps_sum[:], AF.Copy, bias=0.0, scale=1.0 / H)
            sqrow = apool.tile([1, TH], F32, name="ln_sqrow", bufs=1)
            nc.scalar.activation(sqrow[:], ps_sq[:], AF.Copy, bias=0.0, scale=1.0 / H)
            msq = apool.tile([1, TH], F32, name="ln_msq", bufs=1)
            nc.vector.tensor_tensor(out=msq[:], in0=mrow[:], in1=mrow[:], op=OP.mult)
            nc.vector.tensor_tensor(out=sqrow[:], in0=sqrow[:], in1=msq[:], op=OP.subtract)
            nc.vector.tensor_scalar(out=sqrow[:], in0=sqrow[:], scalar1=float(EPS),
                                    scalar2=None, op0=OP.add)
            # rsqrt(v) = exp(-0.5 * ln(v))
            nc.scalar.activation(sqrow[:], sqrow[:], AF.Ln, bias=0.0, scale=1.0)
            rsrow = apool.tile([1, TH], F32, name="ln_rsrow", bufs=1)
            nc.scalar.activation(rsrow[:], sqrow[:], AF.Exp, bias=0.0, scale=-0.5)
            nc.vector.tensor_tensor(out=mrow[:], in0=mrow[:], in1=rsrow[:], op=OP.mult)
            bc_rs = ppool.tile([128, TH], F32, name="psS", bufs=2)
            nc.tensor.matmul(bc_rs[:], onesrow32[:], rsrow[:], start=True, stop=True)
            bc_mrs = ppool.tile([128, TH], F32, name="psS", bufs=2)
            nc.tensor.matmul(bc_mrs[:], onesrow32[:], mrow[:], start=True, stop=True)
            for k in range(HT):
                t = apool.tile([128, TH], F32, name="ln_t", bufs=2)
                nc.vector.tensor_tensor(out=t[:], in0=xt[k][:, t0:t0 + TH].bitcast(F32),
                                        in1=bc_rs[:], op=OP.mult)
                nc.vector.tensor_tensor(out=t[:], in0=t[:], in1=bc_mrs[:], op=OP.subtract)
                nc.scalar.activation(dst_fn(k), t[:], AF.Identity,
                                     bias=bcol[:, k:k + 1], scale=gc[:, k:k + 1])

        def x_update(ag, bias_cc):
            for k in range(HT):
                dst_ = apool.tile([128, S], BF16, name="delta_st", bufs=2)
                nc.sync.dma_start(
                    out=dst_[:],
                    in_=ag[:].rearrange("(a p) t -> p a t", p=128)[:, k, :])
                nc.vector.scalar_tensor_tensor(
                    out=xt[k][:], in0=dst_[:], scalar=bias_cc[:, k:k + 1],
                    in1=xt[k][:].bitcast(F32), op0=OP.add, op1=OP.add)

        def rope_from_psum(ps, biascol, dst):
            """ps [128, S] psum (quadrant-paired layout) -> dst [128, S] f32r rotated."""
            t1 = apool.tile([128, S], F32, name="rope_t", bufs=3)
            nc.vector.scalar_tensor_tensor(out=t1[:], in0=ps[:], scalar=biascol,
                                           in1=ropec[:], op0=OP.add, op1=OP.mult)
            t2 = apool.tile([128, S], F32, name="rope_t", bufs=3)
            nc.gpsimd.scalar_tensor_tensor(out=t2[:], in0=ps[:], scalar=biascol,
                                           in1=ropes[:], op0=OP.add, op1=OP.mult)
            t2s = apool.tile([128, S], F32, name="rope_t", bufs=3)
            nc.vector.stream_shuffle(t2s[:], t2[:], _SWAP16)
            nc.vector.tensor_tensor(out=dst[:], in0=t1[:], in1=t2s[:], op=OP.add)

        for l in range(L):
            if l > 0:
                x_update(agout[l - 1], cc[f"sb2_{l - 1}"])

            # ======================= attention (replicated, 16 heads) =======================
            ht_t = [npool.tile([128, S], F32R, name="nrm", bufs=HT) for _ in range(HT)]
            for th in range(2):
                layer_norm(th, cc[f"ln1g_{l}"], cc[f"ln1b_{l}"],
                           lambda k, _t0=th * TH: ht_t[k][:, _t0:_t0 + TH])

            # kv down-projection: kvt [128, 2, S]
            wdkv = wpool.tile([128, HT, CKV], F32R, name="wdkv")
            nc.sync.dma_start(out=wdkv[:],
                              in_=d[f"wdkv_{l}"][:].rearrange("(a p) c -> p a c", p=128))
            kvt = apool.tile([128, CKV // 128, S], F32R, name="kvt")
            for c in range(CKV // 128):
                ps = ppool.tile([128, S], F32, name="psS", bufs=2)
                for a in range(HT):
                    nc.tensor.matmul(ps[:], wdkv[:, a, ts(c, 128)], ht_t[a][:],
                                     start=(a == 0), stop=(a == HT - 1))
                nc.scalar.activation(kvt[:, c, :], ps[:], AF.Identity,
                                     bias=cc[f"bdkv_{l}"][:, c:c + 1], scale=1.0)

            # q down-projection: qdt [128, 3, S] (frees ht before the head loop)
            qdt = apool.tile([128, CQ // 128, S], F32R, name="qdt")
            for c in range(CQ // 128):
                wdqc = wpool.tile([128, HT, 128], F32R, name="wdqc", bufs=2)
                nc.sync.dma_start(
                    out=wdqc[:],
                    in_=d[f"wdq_{l}"][:].rearrange("(a p) c -> p a c", p=128)[:, :, ts(c, 128)])
                ps = ppool.tile([128, S], F32, name="psS", bufs=2)
                for a in range(HT):
                    nc.tensor.matmul(ps[:], wdqc[:, a, :], ht_t[a][:],
                                     start=(a == 0), stop=(a == HT - 1))
                nc.scalar.activation(qdt[:, c, :], ps[:], AF.Identity,
                                     bias=cc[f"bdq_{l}"][:, c:c + 1], scale=1.0)

            # v up-projection for all heads: vt [128 tok, 4 tl, 1040] (65 cols/head)
            wuv = wpool.tile([128, CKV // 128, NH * HD], F32R, name="wuv")
            nc.sync.dma_start(out=wuv[:],
                              in_=d[f"wuv_{l}"][:].rearrange("(c p) h -> p c h", p=128))
            buvr = wpool.tile([1, NH * HD], F32R, name="buvr")
            nc.sync.dma_start(out=buvr[:], in_=d[f"buvr_{l}"][:])
            vt = apool.tile([128, TT, 65 * NH], F32R, name="vt")
            nc.vector.memset(
                vt[:].rearrange("p tl (h c) -> p tl h c", c=65)[:, :, :, 64:65], 1.0)
            for tl in range(TT):
                for g in range(2):
                    ps = ppool.tile([128, 512], F32, name="psS", bufs=2)
                    nc.tensor.matmul(ps[:], onesrow[:], buvr[:, ts(g, 512)],
                                     start=True, stop=False)
                    for c in range(CKV // 128):
                        nc.tensor.matmul(ps[:], kvt[:, c, ts(tl, 128)],
                                         wuv[:, c, ts(g, 512)],
                                         start=False, stop=(c == CKV // 128 - 1))
                    nc.scalar.copy(
                        vt[:, tl, :].rearrange("p (h c) -> p h c", c=65)[:, 8 * g:8 * g + 8, 0:64],
                        ps[:].rearrange("p (h c) -> p h c", c=64))

            # per head-pair: k/q up-projection + rope + scores + AV + normalize
            aob = []
            ntk = TT
            for hp in range(NHP):
                wukh = wpool.tile([128, CKV // 128, 128], F32R, name="wukh", bufs=2)
                nc.sync.dma_start(
                    out=wukh[:],
                    in_=d[f"wuk_{l}"][:].rearrange("(c p) h -> p c h", p=128)[:, :, ts(hp, 128)])
                ps_k = ppool.tile([128, S], F32, name="psS", bufs=2)
                for c in range(CKV // 128):
                    nc.tensor.matmul(ps_k[:], wukh[:, c, :], kvt[:, c, :],
                                     start=(c == 0), stop=(c == CKV // 128 - 1))
                kfin = apool.tile([128, S], F32R, name="kfin", bufs=2)
                rope_from_psum(ps_k, cc[f"buk_{l}"][:, hp:hp + 1], kfin[:])

                wuqh = wpool.tile([128, CQ // 128, 128], F32R, name="wuqh", bufs=2)
                nc.sync.dma_start(
                    out=wuqh[:],
                    in_=d[f"wuq_{l}"][:].rearrange("(c p) h -> p c h", p=128)[:, :, ts(hp, 128)])
                ps_q = ppool.tile([128, S], F32, name="psS", bufs=2)
                for c in range(CQ // 128):
                    nc.tensor.matmul(ps_q[:], wuqh[:, c, :], qdt[:, c, :],
                                     start=(c == 0), stop=(c == CQ // 128 - 1))
                qfin = apool.tile([128, S], F32R, name="qfin", bufs=2)
                rope_from_psum(ps_q, cc[f"buq_{l}"][:, hp:hp + 1], qfin[:])

                ao_hp = npool.tile([128, S], F32R, name="nrm", bufs=HT)
                for hi in range(2):
                    h = 2 * hp + hi
                    r = 64 * hi
                    av_ps = ppool.tile([65, S], F32, name="psE", bufs=6)
                    for tk in range(ntk):
                        n0 = 128 * tk if causal else 0
                        ps_u = ppool.tile([128, S], F32, name="psE", bufs=6)
                        nc.tensor.matmul(ps_u[:, n0:S], kfin[r:r + 64, ts(tk, 128)],
                                         qfin[r:r + 64, n0:S], start=True, stop=True)
                        if causal:
                            nc.gpsimd.tensor_tensor(out=ps_u[:, n0:n0 + 128],
                                                    in0=ps_u[:, n0:n0 + 128],
                                                    in1=diag[:], op=OP.add)
                        else:
                            nc.gpsimd.tensor_tensor(out=ps_u[:, :], in0=ps_u[:, :],
                                                    in1=maskbt[:, tk, :], op=OP.add)
                        u = apool.tile([128, S], F32R, name="u_exp", bufs=2)
                        nc.scalar.activation(u[:, n0:S], ps_u[:, n0:S], AF.Exp,
                                             bias=0.0, scale=1.0 / np.sqrt(HD))
                        nc.tensor.matmul(av_ps[:, n0:S], vt[:, tk, 65 * h:65 * h + 65],
                                         u[:, n0:S], start=(tk == 0), stop=(tk == ntk - 1),
                                         skip_group_check=True)
                    rrow = apool.tile([1, S], F32, name="rrow", bufs=2)
                    nc.vector.reciprocal(out=rrow[:], in_=av_ps[64:65, :])
                    bc_ps = ppool.tile([64, S], F32, name="psS", bufs=2)
                    nc.tensor.matmul(bc_ps[:], onesrow[:, 0:64], rrow[:].bitcast(F32R),
                                     start=True, stop=True)
                    nc.vector.tensor_tensor(out=ao_hp[r:r + 64, :], in0=av_ps[0:64, :],
                                            in1=bc_ps[:], op=OP.mult)
                aob.append(ao_hp)

            # output projection, added straight into the residual (replicated)
            for hh in range(HT):
                woh = wpool.tile([128, HT, 128], F32R, name="woh", bufs=2)
                nc.sync.dma_start(
                    out=woh[:],
                    in_=d[f"wo_{l}"][:].rearrange("(a p) c -> p a c", p=128)[:, :, ts(hh, 128)])
                ps = ppool.tile([128, S], F32, name="psE", bufs=6)
                for a in range(HT):
                    nc.tensor.matmul(ps[:], woh[:, a, :], aob[a][:],
                                     start=(a == 0), stop=(a == HT - 1))
                nc.vector.scalar_tensor_tensor(
                    out=xt[hh][:], in0=ps[:], scalar=cc[f"bo_{l}"][:, hh:hh + 1],
                    in1=xt[hh][:].bitcast(F32), op0=OP.add, op1=OP.add)

            # ======================= MoE =======================
            hmt = [npool.tile([128, S], F32R, name="nrm", bufs=HT) for _ in range(HT)]
            for th in range(2):
                layer_norm(th, cc[f"ln2g_{l}"], cc[f"ln2b_{l}"],
                           lambda k, _t0=th * TH: hmt[k][:, _t0:_t0 + TH])

            # gate + top-2 weights
            gatew = wpool.tile([128, HT, E], F32R, name="gatew")
            nc.sync.dma_start(out=gatew[:],
                              in_=d[f"gatew_{l}"][:].rearrange("(a p) e -> p a e", p=128))
            gatebr = wpool.tile([1, E], F32R, name="gatebr")
            nc.sync.dma_start(out=gatebr[:], in_=d[f"gatebr_{l}"][:])
            wb = apool.tile([128, S], F32, name="wb", bufs=1)
            for th in range(2):
                t0 = th * TH
                wtok = apool.tile([128, 2, E], F32, name="wtok", bufs=2)
                for tl in range(2):
                    t = 2 * th + tl
                    ps = ppool.tile([128, E], F32, name="psS", bufs=2)
                    nc.tensor.matmul(ps[:], onesrow[:], gatebr[:], start=True, stop=False)
                    for a in range(HT):
                        nc.tensor.matmul(ps[:], hmt[a][:, ts(t, 128)], gatew[:, a, :],
                                         start=False, stop=(a == HT - 1))
                    z = apool.tile([128, E], F32, name="gz", bufs=1)
                    nc.scalar.copy(z[:], ps[:])
                    m1 = apool.tile([128, 1], F32, name="gm1", bufs=2)
                    nc.vector.tensor_reduce(out=m1[:], in_=z[:], axis=mybir.AxisListType.X, op=OP.max)
                    nm1 = apool.tile([128, 1], F32, name="gnm1", bufs=2)
                    nc.vector.tensor_scalar(out=nm1[:], in0=m1[:], scalar1=-1.0,
                                            scalar2=None, op0=OP.mult)
                    eqm = apool.tile([128, E], F32, name="geqm", bufs=1)
                    nc.vector.tensor_scalar(out=eqm[:], in0=z[:], scalar1=m1[:],
                                            scalar2=None, op0=OP.is_ge)
                    zm = apool.tile([128, E], F32, name="gzm", bufs=1)
                    nc.vector.scalar_tensor_tensor(out=zm[:], in0=eqm[:], scalar=-1e30,
                                                   in1=z[:], op0=OP.mult, op1=OP.add)
                    m2 = apool.tile([128, 1], F32, name="gm2", bufs=2)
                    nc.vector.tensor_reduce(out=m2[:], in_=zm[:], axis=mybir.AxisListType.X, op=OP.max)
                    sel = apool.tile([128, E], F32, name="gsel", bufs=1)
                    nc.vector.tensor_scalar(out=sel[:], in0=z[:], scalar1=m2[:],
                                            scalar2=None, op0=OP.is_ge)
                    u8 = apool.tile([128, E], F32, name="gu8", bufs=1)
                    nc.scalar.activation(u8[:], z[:], AF.Exp, bias=nm1[:], scale=1.0)
                    em2 = apool.tile([128, 1], F32, name="gem2", bufs=2)
                    nc.scalar.activation(em2[:], m2[:], AF.Exp, bias=nm1[:], scale=1.0)
                    den = apool.tile([128, 1], F32, name="gden", bufs=2)
                    nc.vector.tensor_scalar(out=den[:], in0=em2[:], scalar1=1.0,
                                            scalar2=None, op0=OP.add)
                    rec = apool.tile([128, 1], F32, name="grec", bufs=2)
                    nc.vector.reciprocal(out=rec[:], in_=den[:])
                    us = apool.tile([128, E], F32, name="gus", bufs=1)
                    nc.vector.tensor_tensor(out=us[:], in0=u8[:], in1=sel[:], op=OP.mult)
                    nc.vector.tensor_scalar(out=wtok[:, tl, :], in0=us[:], scalar1=rec[:],
                                            scalar2=None, op0=OP.mult)
                ps_wt = ppool.tile([E, TH], F32, name="psS", bufs=2)
                for tl in range(2):
                    nc.tensor.transpose(ps_wt[:, ts(tl, 128)], wtok[:, tl, :], ident[:])
                wt_sb = apool.tile([E, TH], F32R, name="wt_sb", bufs=1)
                nc.scalar.copy(wt_sb[:], ps_wt[:])
                ps_wb = ppool.tile([128, TH], F32, name="psS", bufs=2)
                nc.tensor.matmul(ps_wb[:], expsel[:], wt_sb[:], start=True, stop=True)
                nc.scalar.copy(wb[:, t0:t0 + TH], ps_wb[:])

            # expert FFN: W1 streamed per 128-F block -> gelu -> h1; W2 accumulates
            # FG blocks per psum bank in two 4-high output passes.
            moe_acc = apool.tile([128, HT, S], F32, name="moe_acc")
            moesb = apool.tile([128, HT, S], BF16, name="moesb")

            def w1_block(w1_src, b1cc, fb_w, fb_b):
                """One 128-wide F block: W1 matmuls + gelu -> h1 tile [128, S] f32."""
                w1c = wpool.tile([128, HT, 128], F32R, name="w1c", bufs=2)
                nc.sync.dma_start(
                    out=w1c[:],
                    in_=w1_src[:].rearrange("(a p) f -> p a f", p=128)[:, :, ts(fb_w, 128)])
                ps = ppool.tile([128, S], F32, name="psS", bufs=2)
                for a in range(HT):
                    nc.tensor.matmul(ps[:], w1c[:, a, :], hmt[a][:],
                                     start=(a == 0), stop=(a == HT - 1))
                h1 = apool.tile([128, S], F32R, name="h1", bufs=FG + 1)
                nc.scalar.activation(h1[:], ps[:], AF.Gelu,
                                     bias=b1cc[:, fb_b:fb_b + 1], scale=1.0)
                return h1

            def w2_passes(w2_src, nfb, fb0_w, h1s, evac):
                """Two 4-high output passes over nfb F-blocks; evac(hh, acc_ps)."""
                for gp in range(2):
                    accs = [ppool.tile([128, S], F32, name="psE", bufs=6) for _ in range(4)]
                    for i in range(nfb):
                        w2q = wpool.tile([128, 512], F32R, name="w2q", bufs=2)
                        nc.sync.dma_start(
                            out=w2q[:],
                            in_=w2_src[ts(fb0_w + i, 128), ts(gp, 512)])
                        for j in range(4):
                            nc.tensor.matmul(accs[j][:], w2q[:, ts(j, 128)], h1s[i][:],
                                             start=(i == 0), stop=(i == nfb - 1),
                                             skip_group_check=True)
                    for j in range(4):
                        evac(4 * gp + j, accs[j])

            nfg = NFB // FG  # 4 expert F-groups
            for fg in range(nfg):
                h1s = [w1_block(d[f"ew1_{l}"], cc[f"eb1_{l}"], fg * FG + i, fg * FG + i)
                       for i in range(FG)]

                def evac_expert(hh, acc, _fg=fg):
                    if _fg == 0:
                        nc.scalar.activation(moe_acc[:, hh, :], acc[:], AF.Identity,
                                             bias=cc[f"eb2_{l}"][:, hh:hh + 1], scale=1.0)
                    elif _fg % 2 == 1:
                        nc.vector.tensor_tensor(out=moe_acc[:, hh, :],
                                                in0=moe_acc[:, hh, :], in1=acc[:], op=OP.add)
                    else:
                        nc.gpsimd.tensor_tensor(out=moe_acc[:, hh, :],
                                                in0=moe_acc[:, hh, :], in1=acc[:], op=OP.add)

                w2_passes(d[f"ew2_{l}"], FG, fg * FG, h1s, evac_expert)

            # apply top-2 gate weight to the expert output
            for hh in range(HT):
                eng = nc.vector if hh % 2 == 0 else nc.gpsimd
                eng.tensor_tensor(out=moe_acc[:, hh, :], in0=moe_acc[:, hh, :],
                                  in1=wb[:], op=OP.mult)

            # shared expert slice (FSH cols of F) added in
            h1sh = [w1_block(d[f"sw1_{l}"], cc[f"sb1_{l}"], i, i) for i in range(SFB)]

            def evac_shared(hh, acc):
                eng = nc.vector if hh % 2 == 0 else nc.gpsimd
                eng.tensor_tensor(out=moesb[:, hh, :], in0=acc[:],
                                  in1=moe_acc[:, hh, :], op=OP.add)

            w2_passes(d[f"sw2_{l}"], SFB, 0, h1sh, evac_shared)

            # stage out -> ReduceScatter -> AllGather (bf16 delta)
            nc.sync.dma_start(
                out=arin[l][:].rearrange("(a p) t -> p a t", p=128),
                in_=moesb[:])
            nc.gpsimd.collective_compute(
                "ReduceScatter", OP.add, replica_groups=rgroups,
                ins=[arin[l][:]], outs=[rsout[l][:]])
            nc.gpsimd.collective_compute(
                "AllGather", OP.bypass, replica_groups=rgroups,
                ins=[rsout[l][:]], outs=[agout[l][:]])

        # ======================= final LN + LM head =======================
        x_update(agout[L - 1], cc[f"sb2_{L - 1}"])
        xft = [npool.tile([128, S], BF16, name="nrm", bufs=HT) for _ in range(HT)]
        for th in range(2):
            layer_norm(th, cc["flng"], cc["flnb"],
                       lambda k, _t0=th * TH: xft[k][:, _t0:_t0 + TH])

        nvb = (VSH + VB - 1) // VB
        for vb in range(nvb):
            vn = min(VB, VSH - VB * vb)
            owblk = wpool.tile([128, HT, VB], BF16, name="owblk", bufs=2)
            nc.sync.dma_start(
                out=owblk[:, :, 0:vn],
                in_=d["outw"][:].rearrange("(a p) v -> p a v", p=128)[:, :, VB * vb:VB * vb + vn])
            obr = wpool.tile([1, VB], BF16, name="obr", bufs=2)
            nc.sync.dma_start(out=obr[0:1, 0:vn], in_=d["outbr"][0:1, VB * vb:VB * vb + vn])
            for t in range(TT):
                ps = ppool.tile([128, VB], F32, name="psE", bufs=6)
                nc.tensor.matmul(ps[:, 0:vn], onesrowb[:], obr[0:1, 0:vn],
                                 start=True, stop=False)
                for a in range(HT):
                    nc.tensor.matmul(ps[:, 0:vn], xft[a][:, ts(t, 128)],
                                     owblk[:, a, 0:vn], start=False, stop=(a == HT - 1))
                osb = apool.tile([128, VB], F32, name="osb", bufs=2)
                nc.scalar.copy(osb[:, 0:vn], ps[:, 0:vn])
                nc.sync.dma_start(out=out[ts(t, 128), VB * vb:VB * vb + vn],
                                  in_=osb[:, 0:vn])

    nc.compile()
    return nc


def _host_prepare(inputs):
    ids = np.asarray(inputs["input_ids"])[0]
    mask = np.asarray(inputs["attn_mask"])[0, 0]
    causal = bool(np.array_equal(mask, np.tril(np.ones((S, S), dtype=mask.dtype))))

    emb = np.asarray(inputs["emb"], dtype=np.float32)
    x0t = np.ascontiguousarray(emb[ids].T)

    # per-head quadrant-local rope permutation: within each head's 64 dims,
    # [x1(f0..15), x2(f0..15), x1(f16..31), x2(f16..31)]
    ph = np.concatenate([np.arange(0, 32, 2), np.arange(1, 32, 2),
                         np.arange(32, 64, 2), np.arange(33, 64, 2)])
    pcols = np.concatenate([h * HD + ph for h in range(NH)])

    inv = 1.0 / (10000.0 ** (np.arange(0, HD, 2, dtype=np.float32) / HD))
    pos = np.arange(S, dtype=np.float32)
    ang = pos[None, :] * inv[:, None]          # [32 freqs, S]
    cosv, sinv = np.cos(ang), np.sin(ang)
    cA, cB = cosv[0:16], cosv[16:32]
    sA, sB = sinv[0:16], sinv[16:32]
    ropec = np.concatenate([cA, cA, cB, cB, cA, cA, cB, cB]).astype(np.float32)
    ropes = np.concatenate([sA, -sA, sB, -sB, sA, -sA, sB, -sB]).astype(np.float32)

    ii, jj = np.meshgrid(np.arange(128), np.arange(128), indexing="ij")

    def colify(v, width=128):
        v = np.asarray(v, dtype=np.float32).reshape(-1)
        return np.ascontiguousarray(v.reshape(-1, width).T)

    f32a = lambda v: np.ascontiguousarray(np.asarray(v, dtype=np.float32))

    common = {
        "x0t": x0t,
        "ropec": ropec,
        "ropes": ropes,
        "onescol": np.ones((128, 1), np.float32),
        "onesrow": np.ones((1, 128), np.float32),
        "onesrowb": np.ones((1, 128), ml_dtypes.bfloat16),
        "onesrow32": np.ones((1, 128), np.float32),
        "ident": np.eye(128, dtype=np.float32),
        "flng": colify(inputs["fln_g"]), "flnb": colify(inputs["fln_b"]),
    }
    if causal:
        common["diag"] = np.where(ii > jj, np.float32(-1e9), np.float32(0.0))
    else:
        common["maskbt"] = np.where(mask.T == 0, np.float32(-1e9), np.float32(0.0))

    for l in range(L):
        common[f"ln1g_{l}"] = colify(inputs["ln1_g"][l])
        common[f"ln1b_{l}"] = colify(inputs["ln1_b"][l])
        common[f"ln2g_{l}"] = colify(inputs["ln2_g"][l])
        common[f"ln2b_{l}"] = colify(inputs["ln2_b"][l])
        common[f"wdkv_{l}"] = f32a(inputs["Wdkv"][l])
        common[f"bdkv_{l}"] = colify(inputs["bdkv"][l])
        common[f"wuk_{l}"] = f32a(np.asarray(inputs["Wuk"][l])[:, pcols])
        common[f"buk_{l}"] = colify(np.asarray(inputs["buk"][l])[pcols])
        common[f"wdq_{l}"] = f32a(inputs["Wdq"][l])
        common[f"bdq_{l}"] = colify(inputs["bdq"][l])
        common[f"wuq_{l}"] = f32a(np.asarray(inputs["Wuq"][l])[:, pcols])
        common[f"buq_{l}"] = colify(np.asarray(inputs["buq"][l])[pcols])
        common[f"wuv_{l}"] = f32a(inputs["Wuv"][l])
        common[f"buvr_{l}"] = f32a(inputs["buv"][l])[None, :]
        common[f"wo_{l}"] = f32a(inputs["Wo"][l])
        common[f"bo_{l}"] = colify(inputs["bo"][l])
        common[f"gatew_{l}"] = f32a(inputs["gate_W"][l])
        common[f"gatebr_{l}"] = f32a(np.asarray(inputs["gate_b"][l])
                                     + np.asarray(inputs["gate_bias"][l]))[None, :]

    in_maps = []
    for c in range(NCORE):
        m = dict(common)
        m["expsel"] = np.zeros((E, 128), np.float32)
        m["expsel"][c, :] = 1.0
        for l in range(L):
            m[f"sw1_{l}"] = f32a(np.asarray(inputs["sW1"][l])[:, FSH * c:FSH * (c + 1)])
            m[f"sb1_{l}"] = colify(np.asarray(inputs["sb1"][l])[FSH * c:FSH * (c + 1)])
            m[f"sw2_{l}"] = f32a(np.asarray(inputs["sW2"][l])[FSH * c:FSH * (c + 1), :])
            m[f"sb2_{l}"] = colify(inputs["sb2"][l])
            m[f"ew1_{l}"] = f32a(inputs["eW1"][l, c])
            m[f"eb1_{l}"] = colify(inputs["eb1"][l, c])
            m[f"ew2_{l}"] = f32a(inputs["eW2"][l, c])
            m[f"eb2_{l}"] = colify(inputs["eb2"][l, c])
        m["outw"] = f32a(np.asarray(inputs["out_W"])[:, VSH * c:VSH * (c + 1)]).astype(ml_dtypes.bfloat16)
        m["outbr"] = f32a(np.asarray(inputs["out_b"])[VSH * c:VSH * (c + 1)])[None, :].astype(ml_dtypes.bfloat16)
        in_maps.append(m)
    return in_maps, causal


def kernel(**inputs):
    if "attn_mask" not in inputs and "attention_mask" in inputs:
        inputs = dict(inputs)
        inputs["attn_mask"] = inputs.pop("attention_mask")
    in_maps, causal = _host_prepare(inputs)
    if causal not in _CACHE:
        _CACHE[causal] = _build(causal)
    nc = _CACHE[causal]
    res = run_bass_kernel_spmd(nc, in_maps, core_ids=list(range(NCORE)))
    shards = [res.results[c]["out"] for c in range(NCORE)]
    return np.concatenate(shards, axis=1)[None].astype(np.float32)


# revision 21
# speedup vs baseline: 1.1138x; 1.0871x over previous
"""DeepSeekV3-style 2-layer transformer (MLA attention + dense-EP MoE) on 8 Trainium2 cores.

v2 sharding (per core c of 8):
  - Attention fully REPLICATED (all 16 heads on every core): no collective at the
    attention boundary; attention output is added to the local residual directly.
  - MoE expert-parallel: core c runs expert e=c densely over all 512 tokens, scaled
    by the top-2 gate column wtok[:, c] (zero when unselected -> exact); shared
    expert split over the FFN dim (512 cols/core). One ReduceScatter+AllGather
    (bf16 payload) per layer combines the partial deltas -- 2 collectives total.
  - LM head vocab-split (4000 cols/core), concatenated on host.

On-chip layout: residual x kept feature-major [H, T] f32r. LayerNorm via ones-matmul
partition reductions + rank-1 broadcast matmuls. RoPE via host permutation of the
k/q up-projection columns into per-head quadrant-local (x1;x2) pairs so the rotation
is 4 wide ops per 128-row tile (2x STT, stream_shuffle, add). Softmax without max
subtraction; denominators via a ones column in v; per-head reciprocal broadcast by
rank-1 matmul. MoE W2 accumulates 8 F-blocks per psum bank (4 F-groups x two 4-high
output passes) to keep evac traffic low. Matmuls in f32r (moving dim >= 256).
"""

import os
import sys
from contextlib import ExitStack

for _p in ("/opt/trn_rl_repo", "/root/.axon_site/_ro/trn_rl_repo"):
    if os.path.isdir(_p) and _p not in sys.path:
        sys.path.insert(0, _p)

import ml_dtypes
import numpy as np

import concourse.bass as bass
import concourse.tile as tile
from concourse import bacc, mybir
from concourse.bass import ts
from concourse.bass_utils import run_bass_kernel_spmd

F32 = mybir.dt.float32
F32R = mybir.dt.float32r
BF16 = mybir.dt.bfloat16
AF = mybir.ActivationFunctionType
OP = mybir.AluOpType

V, H, NH, HD, CKV, CQ, E, TOPK, L, B, S = 32000, 1024, 16, 64, 256, 384, 8, 2, 2, 1, 512
F = 4 * H
NCORE = 8
VSH = V // NCORE          # 4000
FSH = F // NCORE          # 512
HT = H // 128             # 8 feature tiles
TT = S // 128             # 4 token chunks
NHP = NH // 2             # 8 head-pairs (one 128-row tile each)
EPS = 1e-5
TH = S // 2               # 256
VB = 256                  # vocab block width for LM head
NFB = F // 128            # 32 expert F-blocks
FG = 8                    # F-blocks per W2 accumulation group (1024 F cols)
SFB = FSH // 128          # 4 shared-expert F-blocks

_CACHE = {}

# stream_shuffle mask: swap 16-row halves within each 32-partition quadrant
_SWAP16 = list(range(16, 32)) + list(range(16))


def _build(causal: bool):
    nc = bacc.Bacc("TRN2", target_bir_lowering=False, debug=False, num_devices=NCORE)

    d = {}

    def din(name, shape, dt):
        d[name] = nc.dram_tensor(name, list(shape), dt, kind="ExternalInput")
        return d[name]

    din("x0t", [H, S], F32R)
    din("ropec", [128, S], F32)
    din("ropes", [128, S], F32)
    din("onescol", [128, 1], F32R)
    din("onesrow", [1, 128], F32R)
    din("onesrowb", [1, 128], BF16)
    din("onesrow32", [1, 128], F32)
    din("ident", [128, 128], F32)
    din("expsel", [E, 128], F32R)
    if causal:
        din("diag", [128, 128], F32)
    else:
        din("maskbt", [S, S], F32)
    ccols = []
    for l in range(L):
        for nm, shape in (
            (f"ln1g_{l}", [128, HT]), (f"ln1b_{l}", [128, HT]),
            (f"ln2g_{l}", [128, HT]), (f"ln2b_{l}", [128, HT]),
            (f"bdkv_{l}", [128, CKV // 128]), (f"bdq_{l}", [128, CQ // 128]),
            (f"buk_{l}", [128, NHP]), (f"buq_{l}", [128, NHP]),
            (f"bo_{l}", [128, HT]), (f"sb1_{l}", [128, SFB]),
            (f"sb2_{l}", [128, HT]), (f"eb1_{l}", [128, NFB]),
            (f"eb2_{l}", [128, HT]),
        ):
            din(nm, shape, F32)
            ccols.append(nm)
        din(f"wdkv_{l}", [H, CKV], F32R)
        din(f"wdq_{l}", [H, CQ], F32R)
        din(f"wuk_{l}", [CKV, NH * HD], F32R)
        din(f"wuq_{l}", [CQ, NH * HD], F32R)
        din(f"wuv_{l}", [CKV, NH * HD], F32R)
        din(f"buvr_{l}", [1, NH * HD], F32R)
        din(f"wo_{l}", [NH * HD, H], F32R)
        din(f"gatew_{l}", [H, E], F32R)
        din(f"gatebr_{l}", [1, E], F32R)
        din(f"sw1_{l}", [H, FSH], F32R)
        din(f"sw2_{l}", [FSH, H], F32R)
        din(f"ew1_{l}", [H, F], F32R)
        din(f"ew2_{l}", [F, H], F32R)
    for nm in ("flng", "flnb"):
        din(nm, [128, HT], F32)
        ccols.append(nm)
    din("outw", [H, VSH], BF16)
    din("outbr", [1, VSH], BF16)
    out = nc.dram_tensor("out", [S, VSH], F32, kind="ExternalOutput")

    rgroups = [list(range(NCORE))]

    with ExitStack() as ex, nc.allow_low_precision(reason="f32r matmul pipeline"):
        tc = ex.enter_context(tile.TileContext(nc))
        cpool = ex.enter_context(tc.tile_pool(name="const", bufs=1))
        xpool = ex.enter_context(tc.tile_pool(name="xres", bufs=1))
        npool = ex.enter_context(tc.tile_pool(name="normed", bufs=1))
        wpool = ex.enter_context(tc.tile_pool(name="weights", bufs=1))
        apool = ex.enter_context(tc.tile_pool(name="acts", bufs=1))
        ppool = ex.enter_context(tc.tile_pool(name="psum", bufs=1, space="PSUM"))
        dpool = ex.enter_context(tc.tile_pool(name="dram", bufs=1, space="DRAM"))

        # per-layer collective staging (bf16 delta payloads)
        arin = [dpool.tile([H, S], BF16, name=f"arin{l}") for l in range(L)]
        rsout = [dpool.tile([128, S], BF16, name=f"rsout{l}") for l in range(L)]
        agout = [dpool.tile([H, S], BF16, name=f"agout{l}", addr_space="Shared")
                 for l in range(L)]

        # ---------------- constants ----------------
        def cload(nm, dt=F32):
            t = cpool.tile(list(d[nm].shape), dt, name=f"c_{nm}")
            nc.sync.dma_start(out=t[:], in_=d[nm][:])
            return t

        xt = [xpool.tile([128, S], F32R, name=f"xres_{k}") for k in range(HT)]
        for k in range(HT):
            nc.sync.dma_start(
                out=xt[k][:],
                in_=d["x0t"][:].rearrange("(a p) t -> p a t", p=128)[:, k, :])
        onescol = cload("onescol", F32R)
        onesrow = cload("onesrow", F32R)
        onesrowb = cload("onesrowb", BF16)
        onesrow32 = cload("onesrow32")
        ropec = cload("ropec")
        ropes = cload("ropes")
        ident = cload("ident")
        expsel = cload("expsel", F32R)
        if causal:
            diag = cload("diag")
        else:
            maskbt = cpool.tile([128, TT, S], F32, name="c_maskbt")
            nc.sync.dma_start(out=maskbt[:],
                              in_=d["maskbt"][:].rearrange("(c p) q -> p c q", p=128))
        cc = {nm: cload(nm) for nm in ccols}

        # ---------------- helpers ----------------
        def layer_norm(th, gc, bcol, dst_fn, dst_dt_note=""):
            """Per-token LN of xt token-half th; writes dst_fn(k) <- normalized [128, TH]."""
            t0 = th * TH
            ps_sum = ppool.tile([1, TH], F32, name="psS", bufs=2)
            ps_sq = ppool.tile([1, TH], F32, name="psS", bufs=2)
            for k in range(HT):
                nc.tensor.matmul(ps_sum[:], onescol[:], xt[k][:, t0:t0 + TH],
                                 start=(k == 0), stop=(k == HT - 1))
            for k in range(HT):
                sq = apool.tile([128, TH], F32R, name="ln_sq", bufs=1)
                nc.scalar.square(sq[:], xt[k][:, t0:t0 + TH].bitcast(F32))
                nc.tensor.matmul(ps_sq[:], onescol[:], sq[:],
                                 start=(k == 0), stop=(k == HT - 1))
            mrow = apool.tile([1, TH], F32, name="ln_mrow", bufs=1)
            nc.scalar.activation(mrow[:], ps_sum[:], AF.Copy, bias=0.0, scale=1.0 / H)
            sqrow = apool.tile([1, TH], F32, name="ln_sqrow", bufs=1)
            nc.scalar.activation(sqrow[:], ps_sq[:], AF.Copy, bias=0.0, scale=1.0 / H)
            msq = apool.tile([1, TH], F32, name="ln_msq", bufs=1)
            nc.vector.tensor_tensor(out=msq[:], in0=mrow[:], in1=mrow[:], op=OP.mult)
            nc.vector.tensor_tensor(out=sqrow[:], in0=sqrow[:], in1=msq[:], op=OP.subtract)
            nc.vector.tensor_scalar(out=sqrow[:], in0=sqrow[:], scalar1=float(EPS),
                                    scalar2=None, op0=OP.add)
            # rsqrt(v) = exp(-0.5 * ln(v))
            nc.scalar.activation(sqrow[:], sqrow[:], AF.Ln, bias=0.0, scale=1.0)
            rsrow = apool.tile([1, TH], F32, name="ln_rsrow", bufs=1)
            nc.scalar.activation(rsrow[:], sqrow[:], AF.Exp, bias=0.0, scale=-0.5)
            nc.vector.tensor_tensor(out=mrow[:], in0=mrow[:], in1=rsrow[:], op=OP.mult)
            bc_rs = ppool.tile([128, TH], F32, name="psS", bufs=2)
            nc.tensor.matmul(bc_rs[:], onesrow32[:], rsrow[:], start=True, stop=True)
            bc_mrs = ppool.tile([128, TH], F32, name="psS", bufs=2)
            nc.tensor.matmul(bc_mrs[:], onesrow32[:], mrow[:], start=True, stop=True)
            for k in range(HT):
                t = apool.tile([128, TH], F32, name="ln_t", bufs=2)
                nc.vector.tensor_tensor(out=t[:], in0=xt[k][:, t0:t0 + TH].bitcast(F32),
                                        in1=bc_rs[:], op=OP.mult)
                nc.vector.tensor_tensor(out=t[:], in0=t[:], in1=bc_mrs[:], op=OP.subtract)
                nc.scalar.activation(dst_fn(k), t[:], AF.Identity,
                                     bias=bcol[:, k:k + 1], scale=gc[:, k:k + 1])

        def x_update(ag, bias_cc):
            for k in range(HT):
                dst_ = apool.tile([128, S], BF16, name="delta_st", bufs=2)
                nc.gpsimd.dma_start(
                    out=dst_[:],
                    in_=ag[:].rearrange("(a p) t -> p a t", p=128)[:, k, :])
                nc.vector.scalar_tensor_tensor(
                    out=xt[k][:], in0=dst_[:], scalar=bias_cc[:, k:k + 1],
                    in1=xt[k][:].bitcast(F32), op0=OP.add, op1=OP.add)

        def rope_from_psum(ps, biascol, dst, par):
            """ps [128, S] psum (quadrant-paired layout) -> dst [128, S] f32r rotated."""
            add_eng = nc.gpsimd if par else nc.vector
            t1 = apool.tile([128, S], F32, name="rope_t", bufs=3)
            nc.vector.scalar_tensor_tensor(out=t1[:], in0=ps[:], scalar=biascol,
                                           in1=ropec[:], op0=OP.add, op1=OP.mult)
            t2 = apool.tile([128, S], F32, name="rope_t", bufs=3)
            nc.gpsimd.scalar_tensor_tensor(out=t2[:], in0=ps[:], scalar=biascol,
                                           in1=ropes[:], op0=OP.add, op1=OP.mult)
            t2s = apool.tile([128, S], F32, name="rope_t", bufs=3)
            nc.vector.stream_shuffle(t2s[:], t2[:], _SWAP16)
            add_eng.tensor_tensor(out=dst[:], in0=t1[:], in1=t2s[:], op=OP.add)

        for l in range(L):
            if l > 0:
                x_update(agout[l - 1], cc[f"sb2_{l - 1}"])

            # ======================= attention (replicated, 16 heads) =======================
            ht_t = [npool.tile([128, S], F32R, name="nrm", bufs=HT) for _ in range(HT)]
            for th in range(2):
                layer_norm(th, cc[f"ln1g_{l}"], cc[f"ln1b_{l}"],
                           lambda k, _t0=th * TH: ht_t[k][:, _t0:_t0 + TH])

            # kv down-projection: kvt [128, 2, S]
            wdkv = wpool.tile([128, HT, CKV], F32R, name="wdkv")
            nc.sync.dma_start(out=wdkv[:],
                              in_=d[f"wdkv_{l}"][:].rearrange("(a p) c -> p a c", p=128))
            kvt = apool.tile([128, CKV // 128, S], F32R, name="kvt")
            for c in range(CKV // 128):
                ps = ppool.tile([128, S], F32, name="psS", bufs=2)
                for a in range(HT):
                    nc.tensor.matmul(ps[:], wdkv[:, a, ts(c, 128)], ht_t[a][:],
                                     start=(a == 0), stop=(a == HT - 1))
                nc.scalar.activation(kvt[:, c, :], ps[:], AF.Identity,
                                     bias=cc[f"bdkv_{l}"][:, c:c + 1], scale=1.0)

            # q down-projection: qdt [128, 3, S] (frees ht before the head loop)
            qdt = apool.tile([128, CQ // 128, S], F32R, name="qdt")
            for c in range(CQ // 128):
                wdqc = wpool.tile([128, HT, 128], F32R, name="wdqc", bufs=2)
                nc.sync.dma_start(
                    out=wdqc[:],
                    in_=d[f"wdq_{l}"][:].rearrange("(a p) c -> p a c", p=128)[:, :, ts(c, 128)])
                ps = ppool.tile([128, S], F32, name="psS", bufs=2)
                for a in range(HT):
                    nc.tensor.matmul(ps[:], wdqc[:, a, :], ht_t[a][:],
                                     start=(a == 0), stop=(a == HT - 1))
                nc.scalar.activation(qdt[:, c, :], ps[:], AF.Identity,
                                     bias=cc[f"bdq_{l}"][:, c:c + 1], scale=1.0)

            # v up-projection for all heads: vt [128 tok, 4 tl, 1040] (65 cols/head)
            wuv = wpool.tile([128, CKV // 128, NH * HD], F32R, name="wuv")
            nc.sync.dma_start(out=wuv[:],
                              in_=d[f"wuv_{l}"][:].rearrange("(c p) h -> p c h", p=128))
            buvr = wpool.tile([1, NH * HD], F32R, name="buvr")
            nc.sync.dma_start(out=buvr[:], in_=d[f"buvr_{l}"][:])
            vt = apool.tile([128, TT, 65 * NH], F32R, name="vt")
            nc.vector.memset(
                vt[:].rearrange("p tl (h c) -> p tl h c", c=65)[:, :, :, 64:65], 1.0)
            for tl in range(TT):
                for g in range(2):
                    ps = ppool.tile([128, 512], F32, name="psS", bufs=2)
                    nc.tensor.matmul(ps[:], onesrow[:], buvr[:, ts(g, 512)],
                                     start=True, stop=False)
                    for c in range(CKV // 128):
                        nc.tensor.matmul(ps[:], kvt[:, c, ts(tl, 128)],
                                         wuv[:, c, ts(g, 512)],
                                         start=False, stop=(c == CKV // 128 - 1))
                    nc.scalar.copy(
                        vt[:, tl, :].rearrange("p (h c) -> p h c", c=65)[:, 8 * g:8 * g + 8, 0:64],
                        ps[:].rearrange("p (h c) -> p h c", c=64))

            # per head-pair: k/q up-projection + rope + scores + AV + normalize.
            # kfin/qfin are computed one head-pair AHEAD so rope (DVE/Pool) overlaps
            # the previous pair's scores/AV on PE.
            aob = []
            ntk = TT

            def kq_prep(hp):
                wukh = wpool.tile([128, CKV // 128, 128], F32R, name="wukh", bufs=2)
                nc.sync.dma_start(
                    out=wukh[:],
                    in_=d[f"wuk_{l}"][:].rearrange("(c p) h -> p c h", p=128)[:, :, ts(hp, 128)])
                ps_k = ppool.tile([128, S], F32, name="psS", bufs=2)
                for c in range(CKV // 128):
                    nc.tensor.matmul(ps_k[:], wukh[:, c, :], kvt[:, c, :],
                                     start=(c == 0), stop=(c == CKV // 128 - 1))
                kfin = apool.tile([128, S], F32R, name="kfin", bufs=2)
                rope_from_psum(ps_k, cc[f"buk_{l}"][:, hp:hp + 1], kfin[:], hp % 2)

                wuqh = wpool.tile([128, CQ // 128, 128], F32R, name="wuqh", bufs=2)
                nc.sync.dma_start(
                    out=wuqh[:],
                    in_=d[f"wuq_{l}"][:].rearrange("(c p) h -> p c h", p=128)[:, :, ts(hp, 128)])
                ps_q = ppool.tile([128, S], F32, name="psS", bufs=2)
                for c in range(CQ // 128):
                    nc.tensor.matmul(ps_q[:], wuqh[:, c, :], qdt[:, c, :],
                                     start=(c == 0), stop=(c == CQ // 128 - 1))
                qfin = apool.tile([128, S], F32R, name="qfin", bufs=2)
                rope_from_psum(ps_q, cc[f"buq_{l}"][:, hp:hp + 1], qfin[:], 1 - hp % 2)
                return kfin, qfin

            kq_next = kq_prep(0)
            for hp in range(NHP):
                kfin, qfin = kq_next
                if hp + 1 < NHP:
                    kq_next = kq_prep(hp + 1)

                ao_hp = npool.tile([128, S], F32R, name="nrm", bufs=HT)
                for hi in range(2):
                    h = 2 * hp + hi
                    r = 64 * hi
                    av_ps = ppool.tile([65, S], F32, name="psE", bufs=6)
                    for tk in range(ntk):
                        n0 = 128 * tk if causal else 0
                        ps_u = ppool.tile([128, S], F32, name="psE", bufs=6)
                        nc.tensor.matmul(ps_u[:, n0:S], kfin[r:r + 64, ts(tk, 128)],
                                         qfin[r:r + 64, n0:S], start=True, stop=True)
                        if causal:
                            nc.gpsimd.tensor_tensor(out=ps_u[:, n0:n0 + 128],
                                                    in0=ps_u[:, n0:n0 + 128],
                                                    in1=diag[:], op=OP.add)
                        else:
                            nc.gpsimd.tensor_tensor(out=ps_u[:, :], in0=ps_u[:, :],
                                                    in1=maskbt[:, tk, :], op=OP.add)
                        u = apool.tile([128, S], F32R, name="u_exp", bufs=2)
                        nc.scalar.activation(u[:, n0:S], ps_u[:, n0:S], AF.Exp,
                                             bias=0.0, scale=1.0 / np.sqrt(HD))
                        nc.tensor.matmul(av_ps[:, n0:S], vt[:, tk, 65 * h:65 * h + 65],
                                         u[:, n0:S], start=(tk == 0), stop=(tk == ntk - 1),
                                         skip_group_check=True)
                    rrow = apool.tile([1, S], F32, name="rrow", bufs=2)
                    nc.vector.reciprocal(out=rrow[:], in_=av_ps[64:65, :])
                    bc_ps = ppool.tile([64, S], F32, name="psS", bufs=2)
                    nc.tensor.matmul(bc_ps[:], onesrow[:, 0:64], rrow[:].bitcast(F32R),
                                     start=True, stop=True)
                    nc.vector.tensor_tensor(out=ao_hp[r:r + 64, :], in0=av_ps[0:64, :],
                                            in1=bc_ps[:], op=OP.mult)
                aob.append(ao_hp)

            # output projection, added straight into the residual (replicated)
            for hh in range(HT):
                woh = wpool.tile([128, HT, 128], F32R, name="woh", bufs=2)
                nc.sync.dma_start(
                    out=woh[:],
                    in_=d[f"wo_{l}"][:].rearrange("(a p) c -> p a c", p=128)[:, :, ts(hh, 128)])
                ps = ppool.tile([128, S], F32, name="psE", bufs=6)
                for a in range(HT):
                    nc.tensor.matmul(ps[:], woh[:, a, :], aob[a][:],
                                     start=(a == 0), stop=(a == HT - 1))
                nc.vector.scalar_tensor_tensor(
                    out=xt[hh][:], in0=ps[:], scalar=cc[f"bo_{l}"][:, hh:hh + 1],
                    in1=xt[hh][:].bitcast(F32), op0=OP.add, op1=OP.add)

            # ======================= MoE =======================
            hmt = [npool.tile([128, S], F32R, name="nrm", bufs=HT) for _ in range(HT)]
            for th in range(2):
                layer_norm(th, cc[f"ln2g_{l}"], cc[f"ln2b_{l}"],
                           lambda k, _t0=th * TH: hmt[k][:, _t0:_t0 + TH])

            # gate + top-2 weights
            gatew = wpool.tile([128, HT, E], F32R, name="gatew")
            nc.sync.dma_start(out=gatew[:],
                              in_=d[f"gatew_{l}"][:].rearrange("(a p) e -> p a e", p=128))
            gatebr = wpool.tile([1, E], F32R, name="gatebr")
            nc.sync.dma_start(out=gatebr[:], in_=d[f"gatebr_{l}"][:])
            wb = apool.tile([128, S], F32, name="wb", bufs=1)
            for th in range(2):
                t0 = th * TH
                wtok = apool.tile([128, 2, E], F32, name="wtok", bufs=2)
                for tl in range(2):
                    t = 2 * th + tl
                    ps = ppool.tile([128, E], F32, name="psS", bufs=2)
                    nc.tensor.matmul(ps[:], onesrow[:], gatebr[:], start=True, stop=False)
                    for a in range(HT):
                        nc.tensor.matmul(ps[:], hmt[a][:, ts(t, 128)], gatew[:, a, :],
                                         start=False, stop=(a == HT - 1))
                    z = apool.tile([128, E], F32, name="gz", bufs=1)
                    nc.scalar.copy(z[:], ps[:])
                    m1 = apool.tile([128, 1], F32, name="gm1", bufs=2)
                    nc.vector.tensor_reduce(out=m1[:], in_=z[:], axis=mybir.AxisListType.X, op=OP.max)
                    nm1 = apool.tile([128, 1], F32, name="gnm1", bufs=2)
                    nc.vector.tensor_scalar(out=nm1[:], in0=m1[:], scalar1=-1.0,
                                            scalar2=None, op0=OP.mult)
                    eqm = apool.tile([128, E], F32, name="geqm", bufs=1)
                    nc.vector.tensor_scalar(out=eqm[:], in0=z[:], scalar1=m1[:],
                                            scalar2=None, op0=OP.is_ge)
                    zm = apool.tile([128, E], F32, name="gzm", bufs=1)
                    nc.vector.scalar_tensor_tensor(out=zm[:], in0=eqm[:], scalar=-1e30,
                                                   in1=z[:], op0=OP.mult, op1=OP.add)
                    m2 = apool.tile([128, 1], F32, name="gm2", bufs=2)
                    nc.vector.tensor_reduce(out=m2[:], in_=zm[:], axis=mybir.AxisListType.X, op=OP.max)
                    sel = apool.tile([128, E], F32, name="gsel", bufs=1)
                    nc.vector.tensor_scalar(out=sel[:], in0=z[:], scalar1=m2[:],
                                            scalar2=None, op0=OP.is_ge)
                    u8 = apool.tile([128, E], F32, name="gu8", bufs=1)
                    nc.scalar.activation(u8[:], z[:], AF.Exp, bias=nm1[:], scale=1.0)
                    em2 = apool.tile([128, 1], F32, name="gem2", bufs=2)
                    nc.scalar.activation(em2[:], m2[:], AF.Exp, bias=nm1[:], scale=1.0)
                    den = apool.tile([128, 1], F32, name="gden", bufs=2)
                    nc.vector.tensor_scalar(out=den[:], in0=em2[:], scalar1=1.0,
                                            scalar2=None, op0=OP.add)
                    rec = apool.tile([128, 1], F32, name="grec", bufs=2)
                    nc.vector.reciprocal(out=rec[:], in_=den[:])
                    us = apool.tile([128, E], F32, name="gus", bufs=1)
                    nc.vector.tensor_tensor(out=us[:], in0=u8[:], in1=sel[:], op=OP.mult)
                    nc.vector.tensor_scalar(out=wtok[:, tl, :], in0=us[:], scalar1=rec[:],
                                            scalar2=None, op0=OP.mult)
                ps_wt = ppool.tile([E, TH], F32, name="psS", bufs=2)
                for tl in range(2):
                    nc.tensor.transpose(ps_wt[:, ts(tl, 128)], wtok[:, tl, :], ident[:])
                wt_sb = apool.tile([E, TH], F32R, name="wt_sb", bufs=1)
                nc.scalar.copy(wt_sb[:], ps_wt[:])
                ps_wb = ppool.tile([128, TH], F32, name="psS", bufs=2)
                nc.tensor.matmul(ps_wb[:], expsel[:], wt_sb[:], start=True, stop=True)
                nc.scalar.copy(wb[:, t0:t0 + TH], ps_wb[:])

            # expert FFN: W1 streamed per 128-F block -> gelu -> h1; W2 accumulates
            # FG blocks per psum bank in two 4-high output passes.
            moe_acc = apool.tile([128, HT, S], F32, name="moe_acc")
            moesb = apool.tile([128, HT, S], BF16, name="moesb")

            def w1_block(w1_src, b1cc, fb_w, fb_b):
                """One 128-wide F block: W1 matmuls + gelu -> h1 tile [128, S] f32."""
                w1c = wpool.tile([128, HT, 128], F32R, name="w1c", bufs=2)
                nc.sync.dma_start(
                    out=w1c[:],
                    in_=w1_src[:].rearrange("(a p) f -> p a f", p=128)[:, :, ts(fb_w, 128)])
                ps = ppool.tile([128, S], F32, name="psS", bufs=2)
                for a in range(HT):
                    nc.tensor.matmul(ps[:], w1c[:, a, :], hmt[a][:],
                                     start=(a == 0), stop=(a == HT - 1))
                h1 = apool.tile([128, S], F32R, name="h1", bufs=FG + 1)
                nc.scalar.activation(h1[:], ps[:], AF.Gelu,
                                     bias=b1cc[:, fb_b:fb_b + 1], scale=1.0)
                return h1

            def w2_passes(w2_src, nfb, fb0_w, h1s, evac):
                """Two 4-high output passes over nfb F-blocks; evac(hh, acc_ps)."""
                for gp in range(2):
                    accs = [ppool.tile([128, S], F32, name="psE", bufs=6) for _ in range(4)]
                    for i in range(nfb):
                        w2q = wpool.tile([128, 512], F32R, name="w2q", bufs=2)
                        nc.sync.dma_start(
                            out=w2q[:],
                            in_=w2_src[ts(fb0_w + i, 128), ts(gp, 512)])
                        for j in range(4):
                            nc.tensor.matmul(accs[j][:], w2q[:, ts(j, 128)], h1s[i][:],
                                             start=(i == 0), stop=(i == nfb - 1),
                                             skip_group_check=True)
                    for j in range(4):
                        evac(4 * gp + j, accs[j])

            nfg = NFB // FG  # 4 expert F-groups
            for fg in range(nfg):
                h1s = [w1_block(d[f"ew1_{l}"], cc[f"eb1_{l}"], fg * FG + i, fg * FG + i)
                       for i in range(FG)]

                def evac_expert(hh, acc, _fg=fg):
                    if _fg == 0:
                        nc.scalar.activation(moe_acc[:, hh, :], acc[:], AF.Identity,
                                             bias=cc[f"eb2_{l}"][:, hh:hh + 1], scale=1.0)
                    elif _fg % 2 == 1:
                        nc.vector.tensor_tensor(out=moe_acc[:, hh, :],
                                                in0=moe_acc[:, hh, :], in1=acc[:], op=OP.add)
                    else:
                        nc.gpsimd.tensor_tensor(out=moe_acc[:, hh, :],
                                                in0=moe_acc[:, hh, :], in1=acc[:], op=OP.add)

                w2_passes(d[f"ew2_{l}"], FG, fg * FG, h1s, evac_expert)

            # apply top-2 gate weight to the expert output
            for hh in range(HT):
                eng = nc.vector if hh % 2 == 0 else nc.gpsimd
                eng.tensor_tensor(out=moe_acc[:, hh, :], in0=moe_acc[:, hh, :],
                                  in1=wb[:], op=OP.mult)

            # shared expert slice (FSH cols of F) added in
            h1sh = [w1_block(d[f"sw1_{l}"], cc[f"sb1_{l}"], i, i) for i in range(SFB)]

            def evac_shared(hh, acc):
                eng = nc.vector if hh % 2 == 0 else nc.gpsimd
                eng.tensor_tensor(out=moesb[:, hh, :], in0=acc[:],
                                  in1=moe_acc[:, hh, :], op=OP.add)

            w2_passes(d[f"sw2_{l}"], SFB, 0, h1sh, evac_shared)

            # stage out -> ReduceScatter -> AllGather (bf16 delta)
            nc.gpsimd.dma_start(
                out=arin[l][:].rearrange("(a p) t -> p a t", p=128),
                in_=moesb[:])
            nc.gpsimd.collective_compute(
                "ReduceScatter", OP.add, replica_groups=rgroups,
                ins=[arin[l][:]], outs=[rsout[l][:]])
            nc.gpsimd.collective_compute(
                "AllGather", OP.bypass, replica_groups=rgroups,
                ins=[rsout[l][:]], outs=[agout[l][:]])

        # ======================= final LN + LM head =======================
        x_update(agout[L - 1], cc[f"sb2_{L - 1}"])
        xft = [npool.tile([128, S], BF16, name="nrm", bufs=HT) for _ in range(HT)]
        for th in range(2):
            layer_norm(th, cc["flng"], cc["flnb"],
                       lambda k, _t0=th * TH: xft[k][:, _t0:_t0 + TH])

        nvb = (VSH + VB - 1) // VB
        for vb in range(nvb):
            vn = min(VB, VSH - VB * vb)
            owblk = wpool.tile([128, HT, VB], BF16, name="owblk", bufs=2)
            nc.sync.dma_start(
                out=owblk[:, :, 0:vn],
                in_=d["outw"][:].rearrange("(a p) v -> p a v", p=128)[:, :, VB * vb:VB * vb + vn])
            obr = wpool.tile([1, VB], BF16, name="obr", bufs=2)
            nc.sync.dma_start(out=obr[0:1, 0:vn], in_=d["outbr"][0:1, VB * vb:VB * vb + vn])
            for t in range(TT):
                ps = ppool.tile([128, VB], F32, name="psE", bufs=6)
                nc.tensor.matmul(ps[:, 0:vn], onesrowb[:], obr[0:1, 0:vn],
                                 start=True, stop=False)
                for a in range(HT):
                    nc.tensor.matmul(ps[:, 0:vn], xft[a][:, ts(t, 128)],
                                     owblk[:, a, 0:vn], start=False, stop=(a == HT - 1))
                osb = apool.tile([128, VB], F32, name="osb", bufs=2)
                nc.scalar.copy(osb[:, 0:vn], ps[:, 0:vn])
                nc.scalar.dma_start(out=out[ts(t, 128), VB * vb:VB * vb + vn],
                                    in_=osb[:, 0:vn])

    nc.compile()
    return nc


def _host_prepare(inputs):
    ids = np.asarray(inputs["input_ids"])[0]
    mask = np.asarray(inputs["attn_mask"])[0, 0]
    causal = bool(np.array_equal(mask, np.tril(np.ones((S, S), dtype=mask.dtype))))

    emb = np.asarray(inputs["emb"], dtype=np.float32)
    x0t = np.ascontiguousarray(emb[ids].T)

    # per-head quadrant-local rope permutation: within each head's 64 dims,
    # [x1(f0..15), x2(f0..15), x1(f16..31), x2(f16..31)]
    ph = np.concatenate([np.arange(0, 32, 2), np.arange(1, 32, 2),
                         np.arange(32, 64, 2), np.arange(33, 64, 2)])
    pcols = np.concatenate([h * HD + ph for h in range(NH)])

    inv = 1.0 / (10000.0 ** (np.arange(0, HD, 2, dtype=np.float32) / HD))
    pos = np.arange(S, dtype=np.float32)
    ang = pos[None, :] * inv[:, None]          # [32 freqs, S]
    cosv, sinv = np.cos(ang), np.sin(ang)
    cA, cB = cosv[0:16], cosv[16:32]
    sA, sB = sinv[0:16], sinv[16:32]
    ropec = np.concatenate([cA, cA, cB, cB, cA, cA, cB, cB]).astype(np.float32)
    ropes = np.concatenate([sA, -sA, sB, -sB, sA, -sA, sB, -sB]).astype(np.float32)

    ii, jj = np.meshgrid(np.arange(128), np.arange(128), indexing="ij")

    def colify(v, width=128):
        v = np.asarray(v, dtype=np.float32).reshape(-1)
        return np.ascontiguousarray(v.reshape(-1, width).T)

    f32a = lambda v: np.ascontiguousarray(np.asarray(v, dtype=np.float32))

    common = {
        "x0t": x0t,
        "ropec": ropec,
        "ropes": ropes,
        "onescol": np.ones((128, 1), np.float32),
        "onesrow": np.ones((1, 128), np.float32),
        "onesrowb": np.ones((1, 128), ml_dtypes.bfloat16),
        "onesrow32": np.ones((1, 128), np.float32),
        "ident": np.eye(128, dtype=np.float32),
        "flng": colify(inputs["fln_g"]), "flnb": colify(inputs["fln_b"]),
    }
    if causal:
        common["diag"] = np.where(ii > jj, np.float32(-1e9), np.float32(0.0))
    else:
        common["maskbt"] = np.where(mask.T == 0, np.float32(-1e9), np.float32(0.0))

    for l in range(L):
        common[f"ln1g_{l}"] = colify(inputs["ln1_g"][l])
        common[f"ln1b_{l}"] = colify(inputs["ln1_b"][l])
        common[f"ln2g_{l}"] = colify(inputs["ln2_g"][l])
        common[f"ln2b_{l}"] = colify(inputs["ln2_b"][l])
        common[f"wdkv_{l}"] = f32a(inputs["Wdkv"][l])
        common[f"bdkv_{l}"] = colify(inputs["bdkv"][l])
        common[f"wuk_{l}"] = f32a(np.asarray(inputs["Wuk"][l])[:, pcols])
        common[f"buk_{l}"] = colify(np.asarray(inputs["buk"][l])[pcols])
        common[f"wdq_{l}"] = f32a(inputs["Wdq"][l])
        common[f"bdq_{l}"] = colify(inputs["bdq"][l])
        common[f"wuq_{l}"] = f32a(np.asarray(inputs["Wuq"][l])[:, pcols])
        common[f"buq_{l}"] = colify(np.asarray(inputs["buq"][l])[pcols])
        common[f"wuv_{l}"] = f32a(inputs["Wuv"][l])
        common[f"buvr_{l}"] = f32a(inputs["buv"][l])[None, :]
        common[f"wo_{l}"] = f32a(inputs["Wo"][l])
        common[f"bo_{l}"] = colify(inputs["bo"][l])
        common[f"gatew_{l}"] = f32a(inputs["gate_W"][l])
        common[f"gatebr_{l}"] = f32a(np.asarray(inputs["gate_b"][l])
                                     + np.asarray(inputs["gate_bias"][l]))[None, :]

    in_maps = []
    for c in range(NCORE):
        m = dict(common)
        m["expsel"] = np.zeros((E, 128), np.float32)
        m["expsel"][c, :] = 1.0
        for l in range(L):
            m[f"sw1_{l}"] = f32a(np.asarray(inputs["sW1"][l])[:, FSH * c:FSH * (c + 1)])
            m[f"sb1_{l}"] = colify(np.asarray(inputs["sb1"][l])[FSH * c:FSH * (c + 1)])
            m[f"sw2_{l}"] = f32a(np.asarray(inputs["sW2"][l])[FSH * c:FSH * (c + 1), :])
            m[f"sb2_{l}"] = colify(inputs["sb2"][l])
            m[f"ew1_{l}"] = f32a(inputs["eW1"][l, c])
            m[f"eb1_{l}"] = colify(inputs["eb1"][l, c])
            m[f"ew2_{l}"] = f32a(inputs["eW2"][l, c])
            m[f"eb2_{l}"] = colify(inputs["eb2"][l, c])
        m["outw"] = f32a(np.asarray(inputs["out_W"])[:, VSH * c:VSH * (c + 1)]).astype(ml_dtypes.bfloat16)
        m["outbr"] = f32a(np.asarray(inputs["out_b"])[VSH * c:VSH * (c + 1)])[None, :].astype(ml_dtypes.bfloat16)
        in_maps.append(m)
    return in_maps, causal


def kernel(**inputs):
    if "attn_mask" not in inputs and "attention_mask" in inputs:
        inputs = dict(inputs)
        inputs["attn_mask"] = inputs.pop("attention_mask")
    in_maps, causal = _host_prepare(inputs)
    if causal not in _CACHE:
        _CACHE[causal] = _build(causal)
    nc = _CACHE[causal]
    res = run_bass_kernel_spmd(nc, in_maps, core_ids=list(range(NCORE)))
    shards = [res.results[c]["out"] for c in range(NCORE)]
    return np.concatenate(shards, axis=1)[None].astype(np.float32)
